# revision 42
# baseline (speedup 1.0000x reference)
"""Trainium2 Bass kernel for nn_CWDiscriminator (per-class 3-layer MLP).

reference:
    x = inputs.transpose(0, 2, 1)            # (B, C, F)
    h = relu(einsum('bcf,cfg->bcg', x, W1) + b1)
    h = relu(einsum('bcf,cfg->bcg', h, W2) + b2)
    out = einsum('bcf,cf->bc', h, W3) + b3   # (B, C)

B=16384, F=256, C=19. Data-parallel over B across 8 NeuronCores
(B_loc = 2048 per core). Per core, per class c:
  - inputs arrive as (B_loc, F*C) bf16 (host-cast); PE transpose-mode
    converts the f-strided slices into X.T tiles (f on partitions).
  - GEMM1 (bf16): H1.T = W1[c].T @ X.T  -> PSUM, evicted by ACT with
    fused bias+ReLU to fp32r.
  - GEMM2 (fp32r): H2.T = W2[c].T @ H1.T -> PSUM, evicted with
    bias+ReLU to fp32r (ACT/DVE split).
  - GEMM3 (fp32r): lhsT = W3 masked to column c (128, 19); all classes
    accumulate into one shared PSUM (19, b) region, so the final
    eviction is one op per half instead of per class.
Output per core is (C, B_loc) fp32; host transposes and adds b3.
"""

import sys
import types

import numpy as np
import ml_dtypes

B, F, C = 16384, 256, 19
NCORES = 8
B_LOC = B // NCORES          # 2048
SECTIONS = [512, 512, 512, 512]  # batch columns per PSUM-accum round
assert sum(SECTIONS) == 2048
NCHUNK = 512                 # matmul moving free dim (one fp32 PSUM bank)
FC = F * C                   # 4864

BF16 = ml_dtypes.bfloat16
F8E3 = ml_dtypes.float8_e3m4


# ---------------------------------------------------------------------------
# axon environment shims (NTFF profile hook + artifact upload stub) and the
# one-wait-per-instruction legalizer this walrus build requires.
# ---------------------------------------------------------------------------

def _setup_axon_env():
    if 'antenv.axon_hooks' not in sys.modules:
        mod = types.ModuleType('antenv.axon_hooks')
        mod._hook = None
        mod.set_axon_ntff_profile_hook = lambda h: setattr(mod, '_hook', h)
        mod.get_axon_ntff_profile_hook = lambda: mod._hook
        sys.modules['antenv.axon_hooks'] = mod
        try:
            import antenv
            antenv.axon_hooks = mod
        except ImportError:
            pass
        try:
            from trn_agent_boot.trn_boot import _ntff_profile_via_ctypes
            mod._hook = _ntff_profile_via_ctypes('/opt/axon/libaxon_pjrt.so')
        except Exception:
            pass
    import concourse.bass_utils as bu
    bu.upload_artifacts = lambda tmpdir: 'file://' + str(tmpdir)


def _legalize_waits(nc):
    """walrus accepts at most ONE sync wait per engine instruction (2 for
    EventSemaphore). Split extras onto preceding same-engine NoOps."""
    import concourse.mybir as mybir
    n_split = 0
    for fn in nc.m.functions:
        for bb in fn.blocks:
            insts = bb.instructions
            out = []
            for inst in insts:
                si = inst.sync_info
                ow = list(si.on_wait) if si is not None and si.on_wait else []
                cap = 2 if inst.opcode == "EventSemaphore" else 1
                if len(ow) > cap:
                    keep = ow[-cap:]
                    for k, w in enumerate(ow[:-cap]):
                        nop = mybir.InstNoOp(
                            name=f"{inst.name}-wsplit{k}",
                            engine=inst.engine,
                            ins=[],
                            outs=[],
                            sync_info=mybir.SyncInfo(on_wait=[w], on_update=[]),
                        )
                        out.append(nop)
                        n_split += 1
                    inst.sync_info = mybir.SyncInfo(
                        on_wait=keep,
                        on_update=list(si.on_update) if si.on_update else [],
                    )
                out.append(inst)
            insts[:] = out
    return n_split


# ---------------------------------------------------------------------------
# device program
# ---------------------------------------------------------------------------

_CACHE = {}
last_results = None  # BassKernelResults of the most recent run (for test.py)


def _build_program():
    from contextlib import ExitStack
    import concourse.bass as bass
    import concourse.mybir as mybir
    import concourse.tile as tile

    F32 = mybir.dt.float32
    F32R = mybir.dt.float32r
    B16 = mybir.dt.bfloat16
    F8E3 = mybir.dt.float8e3

    nc = bass.Bass()

    # xt: host-pretransposed input, [p, c, k, b] = x[b, 128k+p, c], bf16
    xtd = nc.declare_dram_parameter("xtd", [128, C, 2, B_LOC], B16,
                                    isOutput=False)
    w1t = nc.declare_dram_parameter("w1t", [128, C, 2, 2, 128], B16,
                                    isOutput=False)
    w2t = nc.declare_dram_parameter("w2t", [128, C * 2 * 2 * 128], B16,
                                    isOutput=False)
    w3m = nc.declare_dram_parameter("w3m", [128, C * 2 * C], B16,
                                    isOutput=False)
    b1s = nc.declare_dram_parameter("b1s", [128, C, 2], F32, isOutput=False)
    b2s = nc.declare_dram_parameter("b2s", [128, C, 2], F32, isOutput=False)
    # out[k, c, b]: partial per k-half; host sums the two halves.
    out = nc.declare_dram_parameter("out", [2, C, B_LOC], F32, isOutput=True)

    NSEC = len(SECTIONS)
    SEC = SECTIONS[0]

    with ExitStack() as ctx:
        tc = ctx.enter_context(tile.TileContext(nc))

        consts = ctx.enter_context(tc.tile_pool(name="consts", bufs=1))
        wtmp_pool = ctx.enter_context(tc.tile_pool(name="wtmp", bufs=1))
        xt_pool = ctx.enter_context(tc.tile_pool(name="xt", bufs=2))
        h1_pool = ctx.enter_context(tc.tile_pool(name="h1p", bufs=3))
        h2_pool = ctx.enter_context(tc.tile_pool(name="h2p", bufs=5))
        out_pool = ctx.enter_context(tc.tile_pool(name="outp", bufs=1))

        ps_g = ctx.enter_context(
            tc.tile_pool(name="ps_g", bufs=6, space="PSUM"))
        # L3 partial-sum banks: k=0 accumulates in col-group 0 (rows 0:19),
        # k=1 in col-group 1 (rows 32:51) of a second bank, so the two
        # narrow matmuls run concurrently on different PE column groups.
        ps_3a = ctx.enter_context(
            tc.tile_pool(name="ps_3a", bufs=1, space="PSUM"))
        ps_3b = ctx.enter_context(
            tc.tile_pool(name="ps_3b", bufs=1, space="PSUM"))

        # ---- X.T section slabs stream on the sync ring, self-paced by
        # the xt pool slots; everything else rides the scalar ring.
        # slab 0 loads immediately (split by class range so L1(c=0) can
        # start early); later slabs are emitted inside the previous
        # section's pipeline, gated on its progress, so their DMA doesn't
        # steal HBM bandwidth from the weight loads at startup.
        xts0 = xt_pool.tile([128, C, 2, SEC], B16, tag="xt")
        # Startup is DMA-latency-bound: the first L1 matmul needs only
        # w1[0:2] (0.26 MB) and x[0:2] (0.5 MB).  Stage tiny first chunks
        # and defer everything not needed in the first few classes so the
        # SDMA rails aren't clogged when the pipeline wants to start.
        nc.sync.dma_start(xts0[:, 0:1], xtd[:, 0:1, :, 0:SEC])
        slabs = [xts0]

        # Weight loads interleaved in class-consumption order: the class-c
        # pipeline needs w1[c] first, w2[c] two iterations later.
        w1sb = consts.tile([128, C, 2, 2, 128], B16)
        w2sb = consts.tile([128, C * 2 * 2 * 128], B16)
        w3sb = consts.tile([128, C * 2 * C], B16)
        b1sb = consts.tile([128, C, 2], F32)
        b2sb = consts.tile([128, C, 2], F32)
        NW2 = C * 2 * 2 * 128  # 9728
        W2C = NW2 // C  # w2 bytes-per-class stride in the flat view

        # Chunk order follows class-consumption time: L1(c) needs w1[c]
        # at step c, L2(c) needs w2[c] at step c+2, the first L3 burst
        # needs w3 at step 6.
        nc.scalar.dma_start(w1sb[:, 0:1], w1t[:, 0:1])
        nc.scalar.dma_start(b1sb[:], b1s[:])
        nc.scalar.dma_start(b2sb[:], b2s[:])
        nc.sync.dma_start(xts0[:, 1:3], xtd[:, 1:3, :, 0:SEC])
        nc.scalar.dma_start(w1sb[:, 1:3], w1t[:, 1:3])
        nc.scalar.dma_start(w2sb[:, 0:1 * W2C], w2t[:, 0:1 * W2C])
        nc.sync.dma_start(xts0[:, 3:6], xtd[:, 3:6, :, 0:SEC])
        nc.scalar.dma_start(w1sb[:, 3:6], w1t[:, 3:6])
        nc.scalar.dma_start(w2sb[:, 1 * W2C:4 * W2C], w2t[:, 1 * W2C:4 * W2C])
        nc.scalar.dma_start(w3sb[:], w3m[:])
        nc.sync.dma_start(xts0[:, 6:10], xtd[:, 6:10, :, 0:SEC])
        nc.scalar.dma_start(w1sb[:, 6:10], w1t[:, 6:10])
        nc.scalar.dma_start(w2sb[:, 4 * W2C:8 * W2C], w2t[:, 4 * W2C:8 * W2C])
        nc.sync.dma_start(xts0[:, 10:C], xtd[:, 10:C, :, 0:SEC])
        nc.scalar.dma_start(w1sb[:, 10:C], w1t[:, 10:C])
        nc.scalar.dma_start(w2sb[:, 8 * W2C:], w2t[:, 8 * W2C:])

        # PE warm-up burst: dummy matmuls while DMA fills SBUF, so the
        # HAM clock-gate reaches 8/8 before the first real GEMM issues.
        wu_l = consts.tile([128, 128], B16)
        nc.gpsimd.memset(wu_l[:], 0.0)
        wu_r = consts.tile([128, 512], B16)
        nc.gpsimd.memset(wu_r[:], 0.0)
        wu_ps = ps_g.tile([128, 512], mybir.dt.float32, tag="pg")
        for i in range(4):
            nc.tensor.matmul(wu_ps[:], wu_l[:], wu_r[:],
                             start=True, stop=True)

        w1v = w1sb[:]
        w2v = w2sb[:].rearrange("p (c k m j) -> p c k m j", c=C, k=2, m=2)
        w3v = w3sb[:].rearrange("p (c k q) -> p c k q", c=C, k=2)

        # Later slabs are allocated and DMA-started from inside the
        # previous section's class loop (gated on pipeline progress via
        # a dummy 1-element write) so their 5 MB transfers don't steal
        # HBM bandwidth from the startup-critical chunks.
        slabs.extend([None] * (NSEC - 1))

        for h in range(NSEC):
            xtv = slabs[h][:]
            sec0 = h * SEC
            ps3a = ps_3a.tile([128, SEC], mybir.dt.float32, tag="ps3a")
            ps3b = ps_3b.tile([128, SEC], mybir.dt.float32, tag="ps3b")
            h1_t = [None, None, None]
            h2_t = [None] * 5
            # Step order on the PE queue: L1(cc), L3-burst, L2(cc-2).
            # L2 lags two steps so h1 is fully evicted well before its
            # matmuls issue.  L3 runs every 4th step as a burst over 4
            # classes (q0 matmuls chained, q32 riding concurrently on
            # the second PE column group / second PSUM bank), so the
            # ~300ns full<->narrow array transition is paid once per
            # four classes instead of once per class.
            for cc in range(C + 4):
                if cc < C:
                    c = cc
                    h1 = h1_pool.tile([128, 2, SEC], B16, tag="h1")
                    h1_t[c % 3] = h1
                    for m in range(2):
                        pg = ps_g.tile([128, SEC], mybir.dt.float32,
                                       tag="pg")
                        for k in range(2):
                            nc.tensor.matmul(
                                pg[:], w1v[:, c, k, m, :],
                                xtv[:, c, k, :],
                                start=(k == 0), stop=(k == 1))
                        if m == 0:
                            nc.scalar.activation(
                                h1[:, m, :], pg[:],
                                mybir.ActivationFunctionType.Relu,
                                bias=b1sb[:, c, m:m+1])
                        else:
                            nc.vector.tensor_scalar(
                                h1[:, m, :], pg[:],
                                b1sb[:, c, m:m+1], 0.0,
                                mybir.AluOpType.add, mybir.AluOpType.max)
                    if cc == 8 and h + 1 < NSEC:
                        xts = xt_pool.tile([128, C, 2, SEC], B16,
                                           tag="xt", name=f"xts{h+1}")
                        # dummy write from h1 delays the slab DMA until
                        # this section is well underway (WAW ordering).
                        nc.gpsimd.tensor_copy(
                            xts[0:1, 0:1, 0:1, 0:1], h1[0:1, 0:1, 0:1])
                        nc.sync.dma_start(
                            xts[:],
                            xtd[:, :, :, (h + 1) * SEC:(h + 2) * SEC])
                        slabs[h + 1] = xts
                burst = {6: (0, 4), 10: (4, 8), 14: (8, 12),
                         18: (12, 16), 20: (16, 18), 21: (18, 19)}
                if cc in burst:
                    for c in range(*burst[cc]):
                        h2 = h2_t[c % 5]
                        nc.tensor.matmul(
                            ps3a[0:C], w3v[:, c, 0, :], h2[:, 0, :],
                            start=(c == 0), stop=(c == C - 1),
                            tile_position=(0, 0))
                        nc.tensor.matmul(
                            ps3b[32:32 + C], w3v[:, c, 1, :],
                            h2[:, 1, :],
                            start=(c == 0), stop=(c == C - 1),
                            tile_position=(0, 32))
                if 2 <= cc <= C + 1:
                    c = cc - 2
                    h1 = h1_t[c % 3]
                    h2 = h2_pool.tile([128, 2, SEC], B16, tag="h2")
                    h2_t[c % 5] = h2
                    for m in range(2):
                        pg = ps_g.tile([128, SEC], mybir.dt.float32,
                                       tag="pg")
                        for k in range(2):
                            nc.tensor.matmul(
                                pg[:], w2v[:, c, k, m, :],
                                h1[:, k, :],
                                start=(k == 0), stop=(k == 1))
                        if m == 0:
                            nc.scalar.activation(
                                h2[:, m, :], pg[:],
                                mybir.ActivationFunctionType.Relu,
                                bias=b2sb[:, c, m:m+1])
                        else:
                            nc.vector.tensor_scalar(
                                h2[:, m, :], pg[:],
                                b2sb[:, c, m:m+1], 0.0,
                                mybir.AluOpType.add, mybir.AluOpType.max)

            # Evict the two L3 partials (k0 at rows 0:19 of bank A, k1
            # at rows 32:51 of bank B); host sums them.  One copy on
            # ACT, one on DVE so neither eviction engine takes both.
            out_sb = out_pool.tile([64, SEC], F32, tag="osb")
            nc.scalar.copy(out_sb[0:C], ps3a[0:C])
            nc.vector.tensor_copy(out_sb[32:32 + C], ps3b[32:32 + C])
            nc.scalar.dma_start(out[0, :, sec0:sec0 + SEC], out_sb[0:C])
            nc.scalar.dma_start(out[1, :, sec0:sec0 + SEC],
                                out_sb[32:32 + C])

    _legalize_waits(nc)
    return nc


def _get_program():
    if 'nc' not in _CACHE:
        _setup_axon_env()
        _CACHE['nc'] = _build_program()
    return _CACHE['nc']


# ---------------------------------------------------------------------------
# host wrapper
# ---------------------------------------------------------------------------

def kernel(inputs, W1, b1, W2, b2, W3, b3):
    global last_results
    from concourse.bass_utils import run_bass_kernel_spmd

    nc = _get_program()

    inputs = np.asarray(inputs)
    W1 = np.asarray(W1, dtype=np.float32)
    b1 = np.asarray(b1, dtype=np.float32)
    W2 = np.asarray(W2, dtype=np.float32)
    b2 = np.asarray(b2, dtype=np.float32)
    W3 = np.asarray(W3, dtype=np.float32)
    b3 = np.asarray(b3, dtype=np.float32)

    # host-side layout prep for the shard: [p, c, k, b] = x[b, 128k+p, c]
    xbf = np.asarray(inputs).reshape(B, 2, 128, C).astype(BF16)
    xtd_full = np.ascontiguousarray(xbf.transpose(2, 3, 1, 0))

    # lhsT tiles: w1t[p, c, k, m, j] = W1[c, 128k+p, 128m+j]
    w1t = np.ascontiguousarray(
        W1.reshape(C, 2, 128, 2, 128).transpose(2, 0, 1, 3, 4)).astype(BF16)
    w2t = np.ascontiguousarray(
        W2.reshape(C, 2, 128, 2, 128).transpose(2, 0, 1, 3, 4)
    ).reshape(128, C * 2 * 2 * 128).astype(BF16)
    # w3m[p, c, k, c'] = (c'==c) * W3[c, 128k+p]
    w3m = np.zeros((128, C, 2, C), dtype=np.float32)
    for c in range(C):
        w3m[:, c, 0, c] = W3[c, :128]
        w3m[:, c, 1, c] = W3[c, 128:]
    w3m = w3m.reshape(128, C * 2 * C).astype(BF16)
    # b1s[p, c, m] = b1[c, 128m+p]
    b1s = np.ascontiguousarray(
        b1.reshape(C, 2, 128).transpose(2, 0, 1)).astype(np.float32)
    b2s = np.ascontiguousarray(
        b2.reshape(C, 2, 128).transpose(2, 0, 1)).astype(np.float32)

    core_ids = list(range(NCORES))
    in_maps = []
    for i in core_ids:
        in_maps.append({
            "xtd": np.ascontiguousarray(
                xtd_full[:, :, :, i * B_LOC:(i + 1) * B_LOC]),
            "w1t": w1t, "w2t": w2t, "w3m": w3m, "b1s": b1s, "b2s": b2s,
        })

    import os
    trace = bool(os.environ.get("BASS_TRACE"))
    res = run_bass_kernel_spmd(nc, in_maps, core_ids, trace=trace)
    last_results = res

    out_full = np.empty((B, C), dtype=np.float32)
    for i in core_ids:
        o2 = res.results[i]["out"]
        out_full[i * B_LOC:(i + 1) * B_LOC] = (o2[0] + o2[1]).T
    out_full += b3[None, :]
    return out_full



# revision 43
# speedup vs baseline: 1.0025x; 1.0025x over previous
"""Trainium2 Bass kernel for nn_CWDiscriminator (per-class 3-layer MLP).

reference:
    x = inputs.transpose(0, 2, 1)            # (B, C, F)
    h = relu(einsum('bcf,cfg->bcg', x, W1) + b1)
    h = relu(einsum('bcf,cfg->bcg', h, W2) + b2)
    out = einsum('bcf,cf->bc', h, W3) + b3   # (B, C)

B=16384, F=256, C=19. Data-parallel over B across 8 NeuronCores
(B_loc = 2048 per core). Per core, per class c:
  - inputs arrive as (B_loc, F*C) bf16 (host-cast); PE transpose-mode
    converts the f-strided slices into X.T tiles (f on partitions).
  - GEMM1 (bf16): H1.T = W1[c].T @ X.T  -> PSUM, evicted by ACT with
    fused bias+ReLU to fp32r.
  - GEMM2 (fp32r): H2.T = W2[c].T @ H1.T -> PSUM, evicted with
    bias+ReLU to fp32r (ACT/DVE split).
  - GEMM3 (fp32r): lhsT = W3 masked to column c (128, 19); all classes
    accumulate into one shared PSUM (19, b) region, so the final
    eviction is one op per half instead of per class.
Output per core is (C, B_loc) fp32; host transposes and adds b3.
"""

import sys
import types

import numpy as np
import ml_dtypes

B, F, C = 16384, 256, 19
NCORES = 8
B_LOC = B // NCORES          # 2048
SECTIONS = [512, 512, 512, 512]  # batch columns per PSUM-accum round
assert sum(SECTIONS) == 2048
NCHUNK = 512                 # matmul moving free dim (one fp32 PSUM bank)
FC = F * C                   # 4864

BF16 = ml_dtypes.bfloat16
F8E3 = ml_dtypes.float8_e3m4


# ---------------------------------------------------------------------------
# axon environment shims (NTFF profile hook + artifact upload stub) and the
# one-wait-per-instruction legalizer this walrus build requires.
# ---------------------------------------------------------------------------

def _setup_axon_env():
    if 'antenv.axon_hooks' not in sys.modules:
        mod = types.ModuleType('antenv.axon_hooks')
        mod._hook = None
        mod.set_axon_ntff_profile_hook = lambda h: setattr(mod, '_hook', h)
        mod.get_axon_ntff_profile_hook = lambda: mod._hook
        sys.modules['antenv.axon_hooks'] = mod
        try:
            import antenv
            antenv.axon_hooks = mod
        except ImportError:
            pass
        try:
            from trn_agent_boot.trn_boot import _ntff_profile_via_ctypes
            mod._hook = _ntff_profile_via_ctypes('/opt/axon/libaxon_pjrt.so')
        except Exception:
            pass
    import concourse.bass_utils as bu
    bu.upload_artifacts = lambda tmpdir: 'file://' + str(tmpdir)


def _legalize_waits(nc):
    """walrus accepts at most ONE sync wait per engine instruction (2 for
    EventSemaphore). Split extras onto preceding same-engine NoOps."""
    import concourse.mybir as mybir
    n_split = 0
    for fn in nc.m.functions:
        for bb in fn.blocks:
            insts = bb.instructions
            out = []
            for inst in insts:
                si = inst.sync_info
                ow = list(si.on_wait) if si is not None and si.on_wait else []
                cap = 2 if inst.opcode == "EventSemaphore" else 1
                if len(ow) > cap:
                    keep = ow[-cap:]
                    for k, w in enumerate(ow[:-cap]):
                        nop = mybir.InstNoOp(
                            name=f"{inst.name}-wsplit{k}",
                            engine=inst.engine,
                            ins=[],
                            outs=[],
                            sync_info=mybir.SyncInfo(on_wait=[w], on_update=[]),
                        )
                        out.append(nop)
                        n_split += 1
                    inst.sync_info = mybir.SyncInfo(
                        on_wait=keep,
                        on_update=list(si.on_update) if si.on_update else [],
                    )
                out.append(inst)
            insts[:] = out
    return n_split


# ---------------------------------------------------------------------------
# device program
# ---------------------------------------------------------------------------

_CACHE = {}
last_results = None  # BassKernelResults of the most recent run (for test.py)


def _build_program():
    from contextlib import ExitStack
    import concourse.bass as bass
    import concourse.mybir as mybir
    import concourse.tile as tile

    F32 = mybir.dt.float32
    F32R = mybir.dt.float32r
    B16 = mybir.dt.bfloat16
    F8E3 = mybir.dt.float8e3

    nc = bass.Bass()

    # xt: host-pretransposed input, [p, c, k, b] = x[b, 128k+p, c], bf16
    xtd = nc.declare_dram_parameter("xtd", [128, C, 2, B_LOC], B16,
                                    isOutput=False)
    w1t = nc.declare_dram_parameter("w1t", [128, C, 2, 2, 128], B16,
                                    isOutput=False)
    w2t = nc.declare_dram_parameter("w2t", [128, C * 2 * 2 * 128], B16,
                                    isOutput=False)
    w3m = nc.declare_dram_parameter("w3m", [128, C * 2 * C], B16,
                                    isOutput=False)
    b1s = nc.declare_dram_parameter("b1s", [128, C, 2], F32, isOutput=False)
    b2s = nc.declare_dram_parameter("b2s", [128, C, 2], F32, isOutput=False)
    # out[k, c, b]: partial per k-half; host sums the two halves.
    out = nc.declare_dram_parameter("out", [2, C, B_LOC], F32, isOutput=True)

    NSEC = len(SECTIONS)
    SEC = SECTIONS[0]

    with ExitStack() as ctx:
        tc = ctx.enter_context(tile.TileContext(nc))

        consts = ctx.enter_context(tc.tile_pool(name="consts", bufs=1))
        wtmp_pool = ctx.enter_context(tc.tile_pool(name="wtmp", bufs=1))
        xt_pool = ctx.enter_context(tc.tile_pool(name="xt", bufs=2))
        h1_pool = ctx.enter_context(tc.tile_pool(name="h1p", bufs=3))
        h2_pool = ctx.enter_context(tc.tile_pool(name="h2p", bufs=5))
        out_pool = ctx.enter_context(tc.tile_pool(name="outp", bufs=1))

        ps_g = ctx.enter_context(
            tc.tile_pool(name="ps_g", bufs=6, space="PSUM"))
        # L3 partial-sum banks: k=0 accumulates in col-group 0 (rows 0:19),
        # k=1 in col-group 1 (rows 32:51) of a second bank, so the two
        # narrow matmuls run concurrently on different PE column groups.
        ps_3a = ctx.enter_context(
            tc.tile_pool(name="ps_3a", bufs=1, space="PSUM"))
        ps_3b = ctx.enter_context(
            tc.tile_pool(name="ps_3b", bufs=1, space="PSUM"))

        # ---- X.T section slabs stream on the sync ring, self-paced by
        # the xt pool slots; everything else rides the scalar ring.
        # slab 0 loads immediately (split by class range so L1(c=0) can
        # start early); later slabs are emitted inside the previous
        # section's pipeline, gated on its progress, so their DMA doesn't
        # steal HBM bandwidth from the weight loads at startup.
        xts0 = xt_pool.tile([128, C, 2, SEC], B16, tag="xt")
        # Startup is DMA-latency-bound: the first L1 matmul needs only
        # w1[0:2] (0.26 MB) and x[0:2] (0.5 MB).  Stage tiny first chunks
        # and defer everything not needed in the first few classes so the
        # SDMA rails aren't clogged when the pipeline wants to start.
        nc.sync.dma_start(xts0[:, 0:1], xtd[:, 0:1, :, 0:SEC])
        slabs = [xts0]

        # Weight loads interleaved in class-consumption order: the class-c
        # pipeline needs w1[c] first, w2[c] two iterations later.
        w1sb = consts.tile([128, C, 2, 2, 128], B16)
        w2sb = consts.tile([128, C * 2 * 2 * 128], B16)
        w3sb = consts.tile([128, C * 2 * C], B16)
        b1sb = consts.tile([128, C, 2], F32)
        b2sb = consts.tile([128, C, 2], F32)
        NW2 = C * 2 * 2 * 128  # 9728
        W2C = NW2 // C  # w2 bytes-per-class stride in the flat view

        nc.scalar.dma_start(w1sb[:, 0:1], w1t[:, 0:1])
        nc.scalar.dma_start(b1sb[:], b1s[:])
        nc.scalar.dma_start(b2sb[:], b2s[:])
        nc.sync.dma_start(xts0[:, 1:3], xtd[:, 1:3, :, 0:SEC])
        nc.scalar.dma_start(w1sb[:, 1:3], w1t[:, 1:3])
        nc.sync.dma_start(xts0[:, 3:7], xtd[:, 3:7, :, 0:SEC])
        nc.scalar.dma_start(w1sb[:, 3:7], w1t[:, 3:7])
        nc.scalar.dma_start(w2sb[:, 0:2 * W2C], w2t[:, 0:2 * W2C])
        nc.scalar.dma_start(w3sb[:], w3m[:])
        nc.sync.dma_start(xts0[:, 7:13], xtd[:, 7:13, :, 0:SEC])
        nc.scalar.dma_start(w1sb[:, 7:13], w1t[:, 7:13])
        nc.scalar.dma_start(w2sb[:, 2 * W2C:7 * W2C], w2t[:, 2 * W2C:7 * W2C])
        nc.sync.dma_start(xts0[:, 13:C], xtd[:, 13:C, :, 0:SEC])
        nc.scalar.dma_start(w1sb[:, 13:C], w1t[:, 13:C])
        nc.scalar.dma_start(w2sb[:, 7 * W2C:], w2t[:, 7 * W2C:])

        # PE warm-up burst: dummy matmuls while DMA fills SBUF, so the
        # HAM clock-gate reaches 8/8 before the first real GEMM issues.
        wu_l = consts.tile([128, 128], B16)
        nc.gpsimd.memset(wu_l[:], 0.0)
        wu_r = consts.tile([128, 512], B16)
        nc.gpsimd.memset(wu_r[:], 0.0)
        wu_ps = ps_g.tile([128, 512], mybir.dt.float32, tag="pg")
        for i in range(4):
            nc.tensor.matmul(wu_ps[:], wu_l[:], wu_r[:],
                             start=True, stop=True)

        w1v = w1sb[:]
        w2v = w2sb[:].rearrange("p (c k m j) -> p c k m j", c=C, k=2, m=2)
        w3v = w3sb[:].rearrange("p (c k q) -> p c k q", c=C, k=2)

        # Later slabs are allocated and DMA-started from inside the
        # previous section's class loop (gated on pipeline progress via
        # a dummy 1-element write) so their 5 MB transfers don't steal
        # HBM bandwidth from the startup-critical chunks.
        slabs.extend([None] * (NSEC - 1))

        for h in range(NSEC):
            xtv = slabs[h][:]
            sec0 = h * SEC
            ps3a = ps_3a.tile([128, SEC], mybir.dt.float32, tag="ps3a")
            ps3b = ps_3b.tile([128, SEC], mybir.dt.float32, tag="ps3b")
            h1_t = [None, None, None]
            h2_t = [None] * 5
            # Step order on the PE queue: L1(cc), L3-burst, L2(cc-2).
            # L2 lags two steps so h1 is fully evicted well before its
            # matmuls issue.  L3 runs every 4th step as a burst over 4
            # classes (q0 matmuls chained, q32 riding concurrently on
            # the second PE column group / second PSUM bank), so the
            # ~300ns full<->narrow array transition is paid once per
            # four classes instead of once per class.
            for cc in range(C + 4):
                if cc < C:
                    c = cc
                    h1 = h1_pool.tile([128, 2, SEC], B16, tag="h1")
                    h1_t[c % 3] = h1
                    for m in range(2):
                        pg = ps_g.tile([128, SEC], mybir.dt.float32,
                                       tag="pg")
                        for k in range(2):
                            nc.tensor.matmul(
                                pg[:], w1v[:, c, k, m, :],
                                xtv[:, c, k, :],
                                start=(k == 0), stop=(k == 1))
                        if m == 0:
                            nc.scalar.activation(
                                h1[:, m, :], pg[:],
                                mybir.ActivationFunctionType.Relu,
                                bias=b1sb[:, c, m:m+1])
                        else:
                            nc.vector.tensor_scalar(
                                h1[:, m, :], pg[:],
                                b1sb[:, c, m:m+1], 0.0,
                                mybir.AluOpType.add, mybir.AluOpType.max)
                    if cc == 8 and h + 1 < NSEC:
                        xts = xt_pool.tile([128, C, 2, SEC], B16,
                                           tag="xt", name=f"xts{h+1}")
                        # dummy write from h1 delays the slab DMA until
                        # this section is well underway (WAW ordering).
                        nc.gpsimd.tensor_copy(
                            xts[0:1, 0:1, 0:1, 0:1], h1[0:1, 0:1, 0:1])
                        nc.sync.dma_start(
                            xts[:],
                            xtd[:, :, :, (h + 1) * SEC:(h + 2) * SEC])
                        slabs[h + 1] = xts
                burst = {6: (0, 4), 10: (4, 8), 14: (8, 12),
                         18: (12, 16), 20: (16, 18), 21: (18, 19)}
                if cc in burst:
                    for c in range(*burst[cc]):
                        h2 = h2_t[c % 5]
                        nc.tensor.matmul(
                            ps3a[0:C], w3v[:, c, 0, :], h2[:, 0, :],
                            start=(c == 0), stop=(c == C - 1),
                            tile_position=(0, 0))
                        nc.tensor.matmul(
                            ps3b[32:32 + C], w3v[:, c, 1, :],
                            h2[:, 1, :],
                            start=(c == 0), stop=(c == C - 1),
                            tile_position=(0, 32))
                if 2 <= cc <= C + 1:
                    c = cc - 2
                    h1 = h1_t[c % 3]
                    h2 = h2_pool.tile([128, 2, SEC], B16, tag="h2")
                    h2_t[c % 5] = h2
                    for m in range(2):
                        pg = ps_g.tile([128, SEC], mybir.dt.float32,
                                       tag="pg")
                        for k in range(2):
                            nc.tensor.matmul(
                                pg[:], w2v[:, c, k, m, :],
                                h1[:, k, :],
                                start=(k == 0), stop=(k == 1))
                        if m == 0:
                            nc.scalar.activation(
                                h2[:, m, :], pg[:],
                                mybir.ActivationFunctionType.Relu,
                                bias=b2sb[:, c, m:m+1])
                        else:
                            nc.vector.tensor_scalar(
                                h2[:, m, :], pg[:],
                                b2sb[:, c, m:m+1], 0.0,
                                mybir.AluOpType.add, mybir.AluOpType.max)

            # Evict the two L3 partials (k0 at rows 0:19 of bank A, k1
            # at rows 32:51 of bank B); host sums them.  One copy on
            # ACT, one on DVE so neither eviction engine takes both.
            out_sb = out_pool.tile([64, SEC], F32, tag="osb")
            nc.scalar.copy(out_sb[0:C], ps3a[0:C])
            nc.vector.tensor_copy(out_sb[32:32 + C], ps3b[32:32 + C])
            nc.scalar.dma_start(out[0, :, sec0:sec0 + SEC], out_sb[0:C])
            nc.scalar.dma_start(out[1, :, sec0:sec0 + SEC],
                                out_sb[32:32 + C])

    _legalize_waits(nc)
    return nc


def _get_program():
    if 'nc' not in _CACHE:
        _setup_axon_env()
        _CACHE['nc'] = _build_program()
    return _CACHE['nc']


# ---------------------------------------------------------------------------
# host wrapper
# ---------------------------------------------------------------------------

def kernel(inputs, W1, b1, W2, b2, W3, b3):
    global last_results
    from concourse.bass_utils import run_bass_kernel_spmd

    nc = _get_program()

    inputs = np.asarray(inputs)
    W1 = np.asarray(W1, dtype=np.float32)
    b1 = np.asarray(b1, dtype=np.float32)
    W2 = np.asarray(W2, dtype=np.float32)
    b2 = np.asarray(b2, dtype=np.float32)
    W3 = np.asarray(W3, dtype=np.float32)
    b3 = np.asarray(b3, dtype=np.float32)

    # host-side layout prep for the shard: [p, c, k, b] = x[b, 128k+p, c]
    xbf = np.asarray(inputs).reshape(B, 2, 128, C).astype(BF16)
    xtd_full = np.ascontiguousarray(xbf.transpose(2, 3, 1, 0))

    # lhsT tiles: w1t[p, c, k, m, j] = W1[c, 128k+p, 128m+j]
    w1t = np.ascontiguousarray(
        W1.reshape(C, 2, 128, 2, 128).transpose(2, 0, 1, 3, 4)).astype(BF16)
    w2t = np.ascontiguousarray(
        W2.reshape(C, 2, 128, 2, 128).transpose(2, 0, 1, 3, 4)
    ).reshape(128, C * 2 * 2 * 128).astype(BF16)
    # w3m[p, c, k, c'] = (c'==c) * W3[c, 128k+p]
    w3m = np.zeros((128, C, 2, C), dtype=np.float32)
    for c in range(C):
        w3m[:, c, 0, c] = W3[c, :128]
        w3m[:, c, 1, c] = W3[c, 128:]
    w3m = w3m.reshape(128, C * 2 * C).astype(BF16)
    # b1s[p, c, m] = b1[c, 128m+p]
    b1s = np.ascontiguousarray(
        b1.reshape(C, 2, 128).transpose(2, 0, 1)).astype(np.float32)
    b2s = np.ascontiguousarray(
        b2.reshape(C, 2, 128).transpose(2, 0, 1)).astype(np.float32)

    core_ids = list(range(NCORES))
    in_maps = []
    for i in core_ids:
        in_maps.append({
            "xtd": np.ascontiguousarray(
                xtd_full[:, :, :, i * B_LOC:(i + 1) * B_LOC]),
            "w1t": w1t, "w2t": w2t, "w3m": w3m, "b1s": b1s, "b2s": b2s,
        })

    import os
    trace = bool(os.environ.get("BASS_TRACE"))
    res = run_bass_kernel_spmd(nc, in_maps, core_ids, trace=trace)
    last_results = res

    out_full = np.empty((B, C), dtype=np.float32)
    for i in core_ids:
        o2 = res.results[i]["out"]
        out_full[i * B_LOC:(i + 1) * B_LOC] = (o2[0] + o2[1]).T
    out_full += b3[None, :]
    return out_full



# revision 44
# speedup vs baseline: 1.0423x; 1.0397x over previous
"""Trainium2 Bass kernel for nn_CWDiscriminator (per-class 3-layer MLP).

reference:
    x = inputs.transpose(0, 2, 1)            # (B, C, F)
    h = relu(einsum('bcf,cfg->bcg', x, W1) + b1)
    h = relu(einsum('bcf,cfg->bcg', h, W2) + b2)
    out = einsum('bcf,cf->bc', h, W3) + b3   # (B, C)

B=16384, F=256, C=19. Data-parallel over B across 8 NeuronCores
(B_loc = 2048 per core). Per core, per class c:
  - inputs arrive as (B_loc, F*C) bf16 (host-cast); PE transpose-mode
    converts the f-strided slices into X.T tiles (f on partitions).
  - GEMM1 (bf16): H1.T = W1[c].T @ X.T  -> PSUM, evicted by ACT with
    fused bias+ReLU to fp32r.
  - GEMM2 (fp32r): H2.T = W2[c].T @ H1.T -> PSUM, evicted with
    bias+ReLU to fp32r (ACT/DVE split).
  - GEMM3 (fp32r): lhsT = W3 masked to column c (128, 19); all classes
    accumulate into one shared PSUM (19, b) region, so the final
    eviction is one op per half instead of per class.
Output per core is (C, B_loc) fp32; host transposes and adds b3.
"""

import sys
import types

import numpy as np
import ml_dtypes

B, F, C = 16384, 256, 19
NCORES = 8
B_LOC = B // NCORES          # 2048
SECTIONS = [512, 512, 512, 512]  # batch columns per PSUM-accum round
assert sum(SECTIONS) == 2048
NCHUNK = 512                 # matmul moving free dim (one fp32 PSUM bank)
FC = F * C                   # 4864

BF16 = ml_dtypes.bfloat16
F8E3 = ml_dtypes.float8_e3m4


# ---------------------------------------------------------------------------
# axon environment shims (NTFF profile hook + artifact upload stub) and the
# one-wait-per-instruction legalizer this walrus build requires.
# ---------------------------------------------------------------------------

def _setup_axon_env():
    if 'antenv.axon_hooks' not in sys.modules:
        mod = types.ModuleType('antenv.axon_hooks')
        mod._hook = None
        mod.set_axon_ntff_profile_hook = lambda h: setattr(mod, '_hook', h)
        mod.get_axon_ntff_profile_hook = lambda: mod._hook
        sys.modules['antenv.axon_hooks'] = mod
        try:
            import antenv
            antenv.axon_hooks = mod
        except ImportError:
            pass
        try:
            from trn_agent_boot.trn_boot import _ntff_profile_via_ctypes
            mod._hook = _ntff_profile_via_ctypes('/opt/axon/libaxon_pjrt.so')
        except Exception:
            pass
    import concourse.bass_utils as bu
    bu.upload_artifacts = lambda tmpdir: 'file://' + str(tmpdir)


def _legalize_waits(nc):
    """walrus accepts at most ONE sync wait per engine instruction (2 for
    EventSemaphore). Split extras onto preceding same-engine NoOps."""
    import concourse.mybir as mybir
    n_split = 0
    for fn in nc.m.functions:
        for bb in fn.blocks:
            insts = bb.instructions
            out = []
            for inst in insts:
                si = inst.sync_info
                ow = list(si.on_wait) if si is not None and si.on_wait else []
                cap = 2 if inst.opcode == "EventSemaphore" else 1
                if len(ow) > cap:
                    keep = ow[-cap:]
                    for k, w in enumerate(ow[:-cap]):
                        nop = mybir.InstNoOp(
                            name=f"{inst.name}-wsplit{k}",
                            engine=inst.engine,
                            ins=[],
                            outs=[],
                            sync_info=mybir.SyncInfo(on_wait=[w], on_update=[]),
                        )
                        out.append(nop)
                        n_split += 1
                    inst.sync_info = mybir.SyncInfo(
                        on_wait=keep,
                        on_update=list(si.on_update) if si.on_update else [],
                    )
                out.append(inst)
            insts[:] = out
    return n_split


# ---------------------------------------------------------------------------
# device program
# ---------------------------------------------------------------------------

_CACHE = {}
last_results = None  # BassKernelResults of the most recent run (for test.py)


def _build_program():
    from contextlib import ExitStack
    import concourse.bass as bass
    import concourse.mybir as mybir
    import concourse.tile as tile

    F32 = mybir.dt.float32
    F32R = mybir.dt.float32r
    B16 = mybir.dt.bfloat16
    F8E3 = mybir.dt.float8e3

    nc = bass.Bass()

    # xt: host-pretransposed input, [p, c, k, b] = x[b, 128k+p, c], bf16
    xtd = nc.declare_dram_parameter("xtd", [128, C, 2, B_LOC], B16,
                                    isOutput=False)
    w1t = nc.declare_dram_parameter("w1t", [128, C, 2, 2, 128], B16,
                                    isOutput=False)
    w2t = nc.declare_dram_parameter("w2t", [128, C * 2 * 2 * 128], B16,
                                    isOutput=False)
    w3m = nc.declare_dram_parameter("w3m", [128, C * 2 * C], B16,
                                    isOutput=False)
    b1s = nc.declare_dram_parameter("b1s", [128, C, 2], F32, isOutput=False)
    b2s = nc.declare_dram_parameter("b2s", [128, C, 2], F32, isOutput=False)
    # out[k, c, b]: partial per k-half; host sums the two halves.
    out = nc.declare_dram_parameter("out", [2, C, B_LOC], F32, isOutput=True)

    NSEC = len(SECTIONS)
    SEC = SECTIONS[0]

    with ExitStack() as ctx:
        tc = ctx.enter_context(tile.TileContext(nc))

        consts = ctx.enter_context(tc.tile_pool(name="consts", bufs=1))
        wtmp_pool = ctx.enter_context(tc.tile_pool(name="wtmp", bufs=1))
        xt_pool = ctx.enter_context(tc.tile_pool(name="xt", bufs=2))
        h1_pool = ctx.enter_context(tc.tile_pool(name="h1p", bufs=3))
        h2_pool = ctx.enter_context(tc.tile_pool(name="h2p", bufs=5))
        out_pool = ctx.enter_context(tc.tile_pool(name="outp", bufs=1))

        ps_g = ctx.enter_context(
            tc.tile_pool(name="ps_g", bufs=6, space="PSUM"))
        # L3 partial-sum banks: k=0 accumulates in col-group 0 (rows 0:19),
        # k=1 in col-group 1 (rows 32:51) of a second bank, so the two
        # narrow matmuls run concurrently on different PE column groups.
        ps_3a = ctx.enter_context(
            tc.tile_pool(name="ps_3a", bufs=1, space="PSUM"))
        ps_3b = ctx.enter_context(
            tc.tile_pool(name="ps_3b", bufs=1, space="PSUM"))

        # ---- X.T section slabs stream on the sync ring, self-paced by
        # the xt pool slots; everything else rides the scalar ring.
        # slab 0 loads immediately (split by class range so L1(c=0) can
        # start early); later slabs are emitted inside the previous
        # section's pipeline, gated on its progress, so their DMA doesn't
        # steal HBM bandwidth from the weight loads at startup.
        xts0 = xt_pool.tile([128, C, 2, SEC], B16, tag="xt")
        # Startup is DMA-latency-bound: the first L1 matmul needs only
        # w1[0:2] (0.26 MB) and x[0:2] (0.5 MB).  Stage tiny first chunks
        # and defer everything not needed in the first few classes so the
        # SDMA rails aren't clogged when the pipeline wants to start.
        nc.sync.dma_start(xts0[:, 0:1], xtd[:, 0:1, :, 0:SEC])
        slabs = [xts0]

        # Weight loads interleaved in class-consumption order: the class-c
        # pipeline needs w1[c] first, w2[c] two iterations later.
        w1sb = consts.tile([128, C, 2, 2, 128], B16)
        w2sb = consts.tile([128, C * 2 * 2 * 128], B16)
        w3sb = consts.tile([128, C * 2 * C], B16)
        b1sb = consts.tile([128, C, 2], F32)
        b2sb = consts.tile([128, C, 2], F32)
        NW2 = C * 2 * 2 * 128  # 9728
        W2C = NW2 // C  # w2 bytes-per-class stride in the flat view

        # All bulk loads ride the sync queue in class-need order; the
        # scalar queue stays free for ACT evictions (each dma_start
        # dispatch costs ~0.7us of engine-queue time).
        nc.sync.dma_start(w1sb[:, 0:1], w1t[:, 0:1])
        nc.sync.dma_start(b1sb[:], b1s[:])
        nc.sync.dma_start(b2sb[:], b2s[:])
        nc.sync.dma_start(xts0[:, 1:3], xtd[:, 1:3, :, 0:SEC])
        nc.sync.dma_start(w1sb[:, 1:3], w1t[:, 1:3])
        nc.sync.dma_start(w2sb[:, 0:1 * W2C], w2t[:, 0:1 * W2C])
        nc.sync.dma_start(xts0[:, 3:7], xtd[:, 3:7, :, 0:SEC])
        nc.sync.dma_start(w1sb[:, 3:7], w1t[:, 3:7])
        nc.sync.dma_start(w2sb[:, 1 * W2C:5 * W2C], w2t[:, 1 * W2C:5 * W2C])
        nc.sync.dma_start(w3sb[:], w3m[:])
        nc.sync.dma_start(xts0[:, 7:13], xtd[:, 7:13, :, 0:SEC])
        nc.sync.dma_start(w1sb[:, 7:13], w1t[:, 7:13])
        nc.sync.dma_start(w2sb[:, 5 * W2C:11 * W2C], w2t[:, 5 * W2C:11 * W2C])
        nc.sync.dma_start(xts0[:, 13:C], xtd[:, 13:C, :, 0:SEC])
        nc.sync.dma_start(w1sb[:, 13:C], w1t[:, 13:C])
        nc.sync.dma_start(w2sb[:, 11 * W2C:], w2t[:, 11 * W2C:])

        # PE warm-up burst: dummy matmuls while DMA fills SBUF, so the
        # HAM clock-gate reaches 8/8 before the first real GEMM issues.
        wu_l = consts.tile([128, 128], B16)
        nc.gpsimd.memset(wu_l[:], 0.0)
        wu_r = consts.tile([128, 512], B16)
        nc.gpsimd.memset(wu_r[:], 0.0)
        wu_ps = ps_g.tile([128, 512], mybir.dt.float32, tag="pg")
        for i in range(4):
            nc.tensor.matmul(wu_ps[:], wu_l[:], wu_r[:],
                             start=True, stop=True)

        w1v = w1sb[:]
        w2v = w2sb[:].rearrange("p (c k m j) -> p c k m j", c=C, k=2, m=2)
        w3v = w3sb[:].rearrange("p (c k q) -> p c k q", c=C, k=2)

        # Later slabs are allocated and DMA-started from inside the
        # previous section's class loop (gated on pipeline progress via
        # a dummy 1-element write) so their 5 MB transfers don't steal
        # HBM bandwidth from the startup-critical chunks.
        slabs.extend([None] * (NSEC - 1))

        for h in range(NSEC):
            xtv = slabs[h][:]
            sec0 = h * SEC
            ps3a = ps_3a.tile([128, SEC], mybir.dt.float32, tag="ps3a")
            ps3b = ps_3b.tile([128, SEC], mybir.dt.float32, tag="ps3b")
            h1_t = [None, None, None]
            h2_t = [None] * 5
            # Step order on the PE queue: L1(cc), L3-burst, L2(cc-2).
            # L2 lags two steps so h1 is fully evicted well before its
            # matmuls issue.  L3 runs every 4th step as a burst over 4
            # classes (q0 matmuls chained, q32 riding concurrently on
            # the second PE column group / second PSUM bank), so the
            # ~300ns full<->narrow array transition is paid once per
            # four classes instead of once per class.
            for cc in range(C + 4):
                if cc < C:
                    c = cc
                    h1 = h1_pool.tile([128, 2, SEC], B16, tag="h1")
                    h1_t[c % 3] = h1
                    for m in range(2):
                        pg = ps_g.tile([128, SEC], mybir.dt.float32,
                                       tag="pg")
                        for k in range(2):
                            nc.tensor.matmul(
                                pg[:], w1v[:, c, k, m, :],
                                xtv[:, c, k, :],
                                start=(k == 0), stop=(k == 1))
                        if m == 0:
                            nc.scalar.activation(
                                h1[:, m, :], pg[:],
                                mybir.ActivationFunctionType.Relu,
                                bias=b1sb[:, c, m:m+1])
                        else:
                            nc.vector.tensor_scalar(
                                h1[:, m, :], pg[:],
                                b1sb[:, c, m:m+1], 0.0,
                                mybir.AluOpType.add, mybir.AluOpType.max)
                    if cc == 8 and h + 1 < NSEC:
                        xts = xt_pool.tile([128, C, 2, SEC], B16,
                                           tag="xt", name=f"xts{h+1}")
                        # dummy write from h1 delays the slab DMA until
                        # this section is well underway (WAW ordering).
                        nc.gpsimd.tensor_copy(
                            xts[0:1, 0:1, 0:1, 0:1], h1[0:1, 0:1, 0:1])
                        nc.sync.dma_start(
                            xts[:],
                            xtd[:, :, :, (h + 1) * SEC:(h + 2) * SEC])
                        slabs[h + 1] = xts
                burst = {6: (0, 4), 10: (4, 8), 14: (8, 12),
                         18: (12, 16), 20: (16, 18), 21: (18, 19)}
                if cc in burst:
                    for c in range(*burst[cc]):
                        h2 = h2_t[c % 5]
                        nc.tensor.matmul(
                            ps3a[0:C], w3v[:, c, 0, :], h2[:, 0, :],
                            start=(c == 0), stop=(c == C - 1),
                            tile_position=(0, 0))
                        nc.tensor.matmul(
                            ps3b[32:32 + C], w3v[:, c, 1, :],
                            h2[:, 1, :],
                            start=(c == 0), stop=(c == C - 1),
                            tile_position=(0, 32))
                if 2 <= cc <= C + 1:
                    c = cc - 2
                    h1 = h1_t[c % 3]
                    h2 = h2_pool.tile([128, 2, SEC], B16, tag="h2")
                    h2_t[c % 5] = h2
                    for m in range(2):
                        pg = ps_g.tile([128, SEC], mybir.dt.float32,
                                       tag="pg")
                        for k in range(2):
                            nc.tensor.matmul(
                                pg[:], w2v[:, c, k, m, :],
                                h1[:, k, :],
                                start=(k == 0), stop=(k == 1))
                        if m == 0:
                            nc.scalar.activation(
                                h2[:, m, :], pg[:],
                                mybir.ActivationFunctionType.Relu,
                                bias=b2sb[:, c, m:m+1])
                        else:
                            nc.vector.tensor_scalar(
                                h2[:, m, :], pg[:],
                                b2sb[:, c, m:m+1], 0.0,
                                mybir.AluOpType.add, mybir.AluOpType.max)

            # Evict the two L3 partials (k0 at rows 0:19 of bank A, k1
            # at rows 32:51 of bank B); host sums them.  One copy on
            # ACT, one on DVE so neither eviction engine takes both.
            out_sb = out_pool.tile([64, SEC], F32, tag="osb")
            nc.scalar.copy(out_sb[0:C], ps3a[0:C])
            nc.vector.tensor_copy(out_sb[32:32 + C], ps3b[32:32 + C])
            nc.sync.dma_start(out[0, :, sec0:sec0 + SEC], out_sb[0:C])
            nc.sync.dma_start(out[1, :, sec0:sec0 + SEC],
                              out_sb[32:32 + C])

    _legalize_waits(nc)
    return nc


def _get_program():
    if 'nc' not in _CACHE:
        _setup_axon_env()
        _CACHE['nc'] = _build_program()
    return _CACHE['nc']


# ---------------------------------------------------------------------------
# host wrapper
# ---------------------------------------------------------------------------

def kernel(inputs, W1, b1, W2, b2, W3, b3):
    global last_results
    from concourse.bass_utils import run_bass_kernel_spmd

    nc = _get_program()

    inputs = np.asarray(inputs)
    W1 = np.asarray(W1, dtype=np.float32)
    b1 = np.asarray(b1, dtype=np.float32)
    W2 = np.asarray(W2, dtype=np.float32)
    b2 = np.asarray(b2, dtype=np.float32)
    W3 = np.asarray(W3, dtype=np.float32)
    b3 = np.asarray(b3, dtype=np.float32)

    # host-side layout prep for the shard: [p, c, k, b] = x[b, 128k+p, c]
    xbf = np.asarray(inputs).reshape(B, 2, 128, C).astype(BF16)
    xtd_full = np.ascontiguousarray(xbf.transpose(2, 3, 1, 0))

    # lhsT tiles: w1t[p, c, k, m, j] = W1[c, 128k+p, 128m+j]
    w1t = np.ascontiguousarray(
        W1.reshape(C, 2, 128, 2, 128).transpose(2, 0, 1, 3, 4)).astype(BF16)
    w2t = np.ascontiguousarray(
        W2.reshape(C, 2, 128, 2, 128).transpose(2, 0, 1, 3, 4)
    ).reshape(128, C * 2 * 2 * 128).astype(BF16)
    # w3m[p, c, k, c'] = (c'==c) * W3[c, 128k+p]
    w3m = np.zeros((128, C, 2, C), dtype=np.float32)
    for c in range(C):
        w3m[:, c, 0, c] = W3[c, :128]
        w3m[:, c, 1, c] = W3[c, 128:]
    w3m = w3m.reshape(128, C * 2 * C).astype(BF16)
    # b1s[p, c, m] = b1[c, 128m+p]
    b1s = np.ascontiguousarray(
        b1.reshape(C, 2, 128).transpose(2, 0, 1)).astype(np.float32)
    b2s = np.ascontiguousarray(
        b2.reshape(C, 2, 128).transpose(2, 0, 1)).astype(np.float32)

    core_ids = list(range(NCORES))
    in_maps = []
    for i in core_ids:
        in_maps.append({
            "xtd": np.ascontiguousarray(
                xtd_full[:, :, :, i * B_LOC:(i + 1) * B_LOC]),
            "w1t": w1t, "w2t": w2t, "w3m": w3m, "b1s": b1s, "b2s": b2s,
        })

    import os
    trace = bool(os.environ.get("BASS_TRACE"))
    res = run_bass_kernel_spmd(nc, in_maps, core_ids, trace=trace)
    last_results = res

    out_full = np.empty((B, C), dtype=np.float32)
    for i in core_ids:
        o2 = res.results[i]["out"]
        out_full[i * B_LOC:(i + 1) * B_LOC] = (o2[0] + o2[1]).T
    out_full += b3[None, :]
    return out_full



# revision 45
# speedup vs baseline: 1.0590x; 1.0161x over previous
"""Trainium2 Bass kernel for nn_CWDiscriminator (per-class 3-layer MLP).

reference:
    x = inputs.transpose(0, 2, 1)            # (B, C, F)
    h = relu(einsum('bcf,cfg->bcg', x, W1) + b1)
    h = relu(einsum('bcf,cfg->bcg', h, W2) + b2)
    out = einsum('bcf,cf->bc', h, W3) + b3   # (B, C)

B=16384, F=256, C=19. Data-parallel over B across 8 NeuronCores
(B_loc = 2048 per core). Per core, per class c:
  - inputs arrive as (B_loc, F*C) bf16 (host-cast); PE transpose-mode
    converts the f-strided slices into X.T tiles (f on partitions).
  - GEMM1 (bf16): H1.T = W1[c].T @ X.T  -> PSUM, evicted by ACT with
    fused bias+ReLU to fp32r.
  - GEMM2 (fp32r): H2.T = W2[c].T @ H1.T -> PSUM, evicted with
    bias+ReLU to fp32r (ACT/DVE split).
  - GEMM3 (fp32r): lhsT = W3 masked to column c (128, 19); all classes
    accumulate into one shared PSUM (19, b) region, so the final
    eviction is one op per half instead of per class.
Output per core is (C, B_loc) fp32; host transposes and adds b3.
"""

import sys
import types

import numpy as np
import ml_dtypes

B, F, C = 16384, 256, 19
NCORES = 8
B_LOC = B // NCORES          # 2048
SECTIONS = [512, 512, 512, 512]  # batch columns per PSUM-accum round
assert sum(SECTIONS) == 2048
NCHUNK = 512                 # matmul moving free dim (one fp32 PSUM bank)
FC = F * C                   # 4864

BF16 = ml_dtypes.bfloat16
F8E3 = ml_dtypes.float8_e3m4


# ---------------------------------------------------------------------------
# axon environment shims (NTFF profile hook + artifact upload stub) and the
# one-wait-per-instruction legalizer this walrus build requires.
# ---------------------------------------------------------------------------

def _setup_axon_env():
    if 'antenv.axon_hooks' not in sys.modules:
        mod = types.ModuleType('antenv.axon_hooks')
        mod._hook = None
        mod.set_axon_ntff_profile_hook = lambda h: setattr(mod, '_hook', h)
        mod.get_axon_ntff_profile_hook = lambda: mod._hook
        sys.modules['antenv.axon_hooks'] = mod
        try:
            import antenv
            antenv.axon_hooks = mod
        except ImportError:
            pass
        try:
            from trn_agent_boot.trn_boot import _ntff_profile_via_ctypes
            mod._hook = _ntff_profile_via_ctypes('/opt/axon/libaxon_pjrt.so')
        except Exception:
            pass
    import concourse.bass_utils as bu
    bu.upload_artifacts = lambda tmpdir: 'file://' + str(tmpdir)


def _legalize_waits(nc):
    """walrus accepts at most ONE sync wait per engine instruction (2 for
    EventSemaphore). Split extras onto preceding same-engine NoOps."""
    import concourse.mybir as mybir
    n_split = 0
    for fn in nc.m.functions:
        for bb in fn.blocks:
            insts = bb.instructions
            out = []
            for inst in insts:
                si = inst.sync_info
                ow = list(si.on_wait) if si is not None and si.on_wait else []
                cap = 2 if inst.opcode == "EventSemaphore" else 1
                if len(ow) > cap:
                    keep = ow[-cap:]
                    for k, w in enumerate(ow[:-cap]):
                        nop = mybir.InstNoOp(
                            name=f"{inst.name}-wsplit{k}",
                            engine=inst.engine,
                            ins=[],
                            outs=[],
                            sync_info=mybir.SyncInfo(on_wait=[w], on_update=[]),
                        )
                        out.append(nop)
                        n_split += 1
                    inst.sync_info = mybir.SyncInfo(
                        on_wait=keep,
                        on_update=list(si.on_update) if si.on_update else [],
                    )
                out.append(inst)
            insts[:] = out
    return n_split


# ---------------------------------------------------------------------------
# device program
# ---------------------------------------------------------------------------

_CACHE = {}
last_results = None  # BassKernelResults of the most recent run (for test.py)


def _build_program():
    from contextlib import ExitStack
    import concourse.bass as bass
    import concourse.mybir as mybir
    import concourse.tile as tile

    F32 = mybir.dt.float32
    F32R = mybir.dt.float32r
    B16 = mybir.dt.bfloat16
    F8E3 = mybir.dt.float8e3

    nc = bass.Bass()

    # xt: host-pretransposed input, [p, c, k, b] = x[b, 128k+p, c], bf16
    xtd = nc.declare_dram_parameter("xtd", [128, C, 2, B_LOC], B16,
                                    isOutput=False)
    w1t = nc.declare_dram_parameter("w1t", [128, C, 2, 2, 128], B16,
                                    isOutput=False)
    w2t = nc.declare_dram_parameter("w2t", [128, C * 2 * 2 * 128], B16,
                                    isOutput=False)
    w3m = nc.declare_dram_parameter("w3m", [128, C * 2 * C], B16,
                                    isOutput=False)
    b1s = nc.declare_dram_parameter("b1s", [128, C, 2], F32, isOutput=False)
    b2s = nc.declare_dram_parameter("b2s", [128, C, 2], F32, isOutput=False)
    # out[k, c, b]: partial per k-half; host sums the two halves.
    out = nc.declare_dram_parameter("out", [2, C, B_LOC], F32, isOutput=True)

    NSEC = len(SECTIONS)
    SEC = SECTIONS[0]

    with ExitStack() as ctx:
        tc = ctx.enter_context(tile.TileContext(nc))

        consts = ctx.enter_context(tc.tile_pool(name="consts", bufs=1))
        wtmp_pool = ctx.enter_context(tc.tile_pool(name="wtmp", bufs=1))
        xt_pool = ctx.enter_context(tc.tile_pool(name="xt", bufs=2))
        h1_pool = ctx.enter_context(tc.tile_pool(name="h1p", bufs=3))
        h2_pool = ctx.enter_context(tc.tile_pool(name="h2p", bufs=5))
        out_pool = ctx.enter_context(tc.tile_pool(name="outp", bufs=1))

        ps_g = ctx.enter_context(
            tc.tile_pool(name="ps_g", bufs=6, space="PSUM"))
        # L3 partial-sum banks: k=0 accumulates in col-group 0 (rows 0:19),
        # k=1 in col-group 1 (rows 32:51) of a second bank, so the two
        # narrow matmuls run concurrently on different PE column groups.
        ps_3a = ctx.enter_context(
            tc.tile_pool(name="ps_3a", bufs=1, space="PSUM"))
        ps_3b = ctx.enter_context(
            tc.tile_pool(name="ps_3b", bufs=1, space="PSUM"))

        # ---- X.T section slabs stream on the sync ring, self-paced by
        # the xt pool slots; everything else rides the scalar ring.
        # slab 0 loads immediately (split by class range so L1(c=0) can
        # start early); later slabs are emitted inside the previous
        # section's pipeline, gated on its progress, so their DMA doesn't
        # steal HBM bandwidth from the weight loads at startup.
        xts0 = xt_pool.tile([128, C, 2, SEC], B16, tag="xt")
        # Startup is DMA-latency-bound: the first L1 matmul needs only
        # w1[0:2] (0.26 MB) and x[0:2] (0.5 MB).  Stage tiny first chunks
        # and defer everything not needed in the first few classes so the
        # SDMA rails aren't clogged when the pipeline wants to start.
        nc.sync.dma_start(xts0[:, 0:1], xtd[:, 0:1, :, 0:SEC])
        slabs = [xts0]

        # Weight loads interleaved in class-consumption order: the class-c
        # pipeline needs w1[c] first, w2[c] two iterations later.
        w1sb = consts.tile([128, C, 2, 2, 128], B16)
        w2sb = consts.tile([128, C * 2 * 2 * 128], B16)
        w3sb = consts.tile([128, C * 2 * C], B16)
        b1sb = consts.tile([128, C, 2], F32)
        b2sb = consts.tile([128, C, 2], F32)
        NW2 = C * 2 * 2 * 128  # 9728
        W2C = NW2 // C  # w2 bytes-per-class stride in the flat view

        # All bulk loads ride the sync queue in class-need order; the
        # scalar queue stays free for ACT evictions (each dma_start
        # dispatch costs ~0.7us of engine-queue time).
        nc.sync.dma_start(w1sb[:, 0:1], w1t[:, 0:1])
        nc.sync.dma_start(b1sb[:], b1s[:])
        nc.sync.dma_start(b2sb[:], b2s[:])
        nc.sync.dma_start(xts0[:, 1:3], xtd[:, 1:3, :, 0:SEC])
        nc.sync.dma_start(w1sb[:, 1:3], w1t[:, 1:3])
        nc.sync.dma_start(w2sb[:, 0:1 * W2C], w2t[:, 0:1 * W2C])
        nc.sync.dma_start(xts0[:, 3:7], xtd[:, 3:7, :, 0:SEC])
        nc.sync.dma_start(w1sb[:, 3:7], w1t[:, 3:7])
        nc.sync.dma_start(w2sb[:, 1 * W2C:5 * W2C], w2t[:, 1 * W2C:5 * W2C])
        nc.sync.dma_start(w3sb[:], w3m[:])
        nc.sync.dma_start(xts0[:, 7:13], xtd[:, 7:13, :, 0:SEC])
        nc.sync.dma_start(w1sb[:, 7:13], w1t[:, 7:13])
        nc.sync.dma_start(w2sb[:, 5 * W2C:11 * W2C], w2t[:, 5 * W2C:11 * W2C])
        nc.sync.dma_start(xts0[:, 13:C], xtd[:, 13:C, :, 0:SEC])
        nc.sync.dma_start(w1sb[:, 13:C], w1t[:, 13:C])
        nc.sync.dma_start(w2sb[:, 11 * W2C:], w2t[:, 11 * W2C:])

        # PE warm-up burst: dummy matmuls while DMA fills SBUF, so the
        # HAM clock-gate reaches 8/8 before the first real GEMM issues.
        wu_l = consts.tile([128, 128], B16)
        nc.gpsimd.memset(wu_l[:], 0.0)
        wu_r = consts.tile([128, 512], B16)
        nc.gpsimd.memset(wu_r[:], 0.0)
        wu_ps = ps_g.tile([128, 512], mybir.dt.float32, tag="pg")
        for i in range(4):
            nc.tensor.matmul(wu_ps[:], wu_l[:], wu_r[:],
                             start=True, stop=True)

        w1v = w1sb[:]
        w2v = w2sb[:].rearrange("p (c k m j) -> p c k m j", c=C, k=2, m=2)
        w3v = w3sb[:].rearrange("p (c k q) -> p c k q", c=C, k=2)

        # Later slabs are allocated and DMA-started from inside the
        # previous section's class loop (gated on pipeline progress via
        # a dummy 1-element write) so their 5 MB transfers don't steal
        # HBM bandwidth from the startup-critical chunks.
        slabs.extend([None] * (NSEC - 1))

        for h in range(NSEC):
            xtv = slabs[h][:]
            sec0 = h * SEC
            ps3a = ps_3a.tile([128, SEC], mybir.dt.float32, tag="ps3a")
            ps3b = ps_3b.tile([128, SEC], mybir.dt.float32, tag="ps3b")
            h1_t = [None, None, None]
            h2_t = [None] * 5
            # Step order on the PE queue: L1(cc), L3-burst, L2(cc-2).
            # L2 lags two steps so h1 is fully evicted well before its
            # matmuls issue.  L3 runs every 4th step as a burst over 4
            # classes (q0 matmuls chained, q32 riding concurrently on
            # the second PE column group / second PSUM bank), so the
            # ~300ns full<->narrow array transition is paid once per
            # four classes instead of once per class.
            for cc in range(C + 4):
                if cc < C:
                    c = cc
                    h1 = h1_pool.tile([128, 2, SEC], B16, tag="h1")
                    h1_t[c % 3] = h1
                    for m in range(2):
                        pg = ps_g.tile([128, SEC], mybir.dt.float32,
                                       tag="pg")
                        for k in range(2):
                            nc.tensor.matmul(
                                pg[:], w1v[:, c, k, m, :],
                                xtv[:, c, k, :],
                                start=(k == 0), stop=(k == 1))
                        if m == 0:
                            nc.scalar.activation(
                                h1[:, m, :], pg[:],
                                mybir.ActivationFunctionType.Relu,
                                bias=b1sb[:, c, m:m+1])
                        else:
                            nc.vector.tensor_scalar(
                                h1[:, m, :], pg[:],
                                b1sb[:, c, m:m+1], 0.0,
                                mybir.AluOpType.add, mybir.AluOpType.max)
                    if cc == 8 and h + 1 < NSEC:
                        xts = xt_pool.tile([128, C, 2, SEC], B16,
                                           tag="xt", name=f"xts{h+1}")
                        # dummy write from h1 delays the slab DMA until
                        # this section is well underway (WAW ordering).
                        nc.gpsimd.tensor_copy(
                            xts[0:1, 0:1, 0:1, 0:1], h1[0:1, 0:1, 0:1])
                        nc.sync.dma_start(
                            xts[:],
                            xtd[:, :, :, (h + 1) * SEC:(h + 2) * SEC])
                        slabs[h + 1] = xts
                burst = {6: (0, 4), 10: (4, 8), 14: (8, 12),
                         18: (12, 16), 21: (16, 19)}
                if cc in burst:
                    for c in range(*burst[cc]):
                        h2 = h2_t[c % 5]
                        nc.tensor.matmul(
                            ps3a[0:C], w3v[:, c, 0, :], h2[:, 0, :],
                            start=(c == 0), stop=(c == C - 1),
                            tile_position=(0, 0))
                        nc.tensor.matmul(
                            ps3b[32:32 + C], w3v[:, c, 1, :],
                            h2[:, 1, :],
                            start=(c == 0), stop=(c == C - 1),
                            tile_position=(0, 32))
                if 2 <= cc <= C + 1:
                    c = cc - 2
                    h1 = h1_t[c % 3]
                    h2 = h2_pool.tile([128, 2, SEC], B16, tag="h2")
                    h2_t[c % 5] = h2
                    for m in range(2):
                        pg = ps_g.tile([128, SEC], mybir.dt.float32,
                                       tag="pg")
                        for k in range(2):
                            nc.tensor.matmul(
                                pg[:], w2v[:, c, k, m, :],
                                h1[:, k, :],
                                start=(k == 0), stop=(k == 1))
                        if m == 0:
                            nc.scalar.activation(
                                h2[:, m, :], pg[:],
                                mybir.ActivationFunctionType.Relu,
                                bias=b2sb[:, c, m:m+1])
                        else:
                            nc.vector.tensor_scalar(
                                h2[:, m, :], pg[:],
                                b2sb[:, c, m:m+1], 0.0,
                                mybir.AluOpType.add, mybir.AluOpType.max)

            # Evict the two L3 partials (k0 at rows 0:19 of bank A, k1
            # at rows 32:51 of bank B); host sums them.  One copy on
            # ACT, one on DVE so neither eviction engine takes both.
            out_sb = out_pool.tile([64, SEC], F32, tag="osb")
            nc.scalar.copy(out_sb[0:C], ps3a[0:C])
            nc.vector.tensor_copy(out_sb[32:32 + C], ps3b[32:32 + C])
            nc.sync.dma_start(out[0, :, sec0:sec0 + SEC], out_sb[0:C])
            nc.sync.dma_start(out[1, :, sec0:sec0 + SEC],
                              out_sb[32:32 + C])

    _legalize_waits(nc)
    return nc


def _get_program():
    if 'nc' not in _CACHE:
        _setup_axon_env()
        _CACHE['nc'] = _build_program()
    return _CACHE['nc']


# ---------------------------------------------------------------------------
# host wrapper
# ---------------------------------------------------------------------------

def kernel(inputs, W1, b1, W2, b2, W3, b3):
    global last_results
    from concourse.bass_utils import run_bass_kernel_spmd

    nc = _get_program()

    inputs = np.asarray(inputs)
    W1 = np.asarray(W1, dtype=np.float32)
    b1 = np.asarray(b1, dtype=np.float32)
    W2 = np.asarray(W2, dtype=np.float32)
    b2 = np.asarray(b2, dtype=np.float32)
    W3 = np.asarray(W3, dtype=np.float32)
    b3 = np.asarray(b3, dtype=np.float32)

    # host-side layout prep for the shard: [p, c, k, b] = x[b, 128k+p, c]
    xbf = np.asarray(inputs).reshape(B, 2, 128, C).astype(BF16)
    xtd_full = np.ascontiguousarray(xbf.transpose(2, 3, 1, 0))

    # lhsT tiles: w1t[p, c, k, m, j] = W1[c, 128k+p, 128m+j]
    w1t = np.ascontiguousarray(
        W1.reshape(C, 2, 128, 2, 128).transpose(2, 0, 1, 3, 4)).astype(BF16)
    w2t = np.ascontiguousarray(
        W2.reshape(C, 2, 128, 2, 128).transpose(2, 0, 1, 3, 4)
    ).reshape(128, C * 2 * 2 * 128).astype(BF16)
    # w3m[p, c, k, c'] = (c'==c) * W3[c, 128k+p]
    w3m = np.zeros((128, C, 2, C), dtype=np.float32)
    for c in range(C):
        w3m[:, c, 0, c] = W3[c, :128]
        w3m[:, c, 1, c] = W3[c, 128:]
    w3m = w3m.reshape(128, C * 2 * C).astype(BF16)
    # b1s[p, c, m] = b1[c, 128m+p]
    b1s = np.ascontiguousarray(
        b1.reshape(C, 2, 128).transpose(2, 0, 1)).astype(np.float32)
    b2s = np.ascontiguousarray(
        b2.reshape(C, 2, 128).transpose(2, 0, 1)).astype(np.float32)

    core_ids = list(range(NCORES))
    in_maps = []
    for i in core_ids:
        in_maps.append({
            "xtd": np.ascontiguousarray(
                xtd_full[:, :, :, i * B_LOC:(i + 1) * B_LOC]),
            "w1t": w1t, "w2t": w2t, "w3m": w3m, "b1s": b1s, "b2s": b2s,
        })

    import os
    trace = bool(os.environ.get("BASS_TRACE"))
    res = run_bass_kernel_spmd(nc, in_maps, core_ids, trace=trace)
    last_results = res

    out_full = np.empty((B, C), dtype=np.float32)
    for i in core_ids:
        o2 = res.results[i]["out"]
        out_full[i * B_LOC:(i + 1) * B_LOC] = (o2[0] + o2[1]).T
    out_full += b3[None, :]
    return out_full



# revision 46
# speedup vs baseline: 1.0776x; 1.0176x over previous
"""Trainium2 Bass kernel for nn_CWDiscriminator (per-class 3-layer MLP).

reference:
    x = inputs.transpose(0, 2, 1)            # (B, C, F)
    h = relu(einsum('bcf,cfg->bcg', x, W1) + b1)
    h = relu(einsum('bcf,cfg->bcg', h, W2) + b2)
    out = einsum('bcf,cf->bc', h, W3) + b3   # (B, C)

B=16384, F=256, C=19. Data-parallel over B across 8 NeuronCores
(B_loc = 2048 per core). Per core, per class c:
  - inputs arrive as (B_loc, F*C) bf16 (host-cast); PE transpose-mode
    converts the f-strided slices into X.T tiles (f on partitions).
  - GEMM1 (bf16): H1.T = W1[c].T @ X.T  -> PSUM, evicted by ACT with
    fused bias+ReLU to fp32r.
  - GEMM2 (fp32r): H2.T = W2[c].T @ H1.T -> PSUM, evicted with
    bias+ReLU to fp32r (ACT/DVE split).
  - GEMM3 (fp32r): lhsT = W3 masked to column c (128, 19); all classes
    accumulate into one shared PSUM (19, b) region, so the final
    eviction is one op per half instead of per class.
Output per core is (C, B_loc) fp32; host transposes and adds b3.
"""

import sys
import types

import numpy as np
import ml_dtypes

B, F, C = 16384, 256, 19
NCORES = 8
B_LOC = B // NCORES          # 2048
SECTIONS = [512, 512, 512, 512]  # batch columns per PSUM-accum round
assert sum(SECTIONS) == 2048
NCHUNK = 512                 # matmul moving free dim (one fp32 PSUM bank)
FC = F * C                   # 4864

BF16 = ml_dtypes.bfloat16
F8E3 = ml_dtypes.float8_e3m4


# ---------------------------------------------------------------------------
# axon environment shims (NTFF profile hook + artifact upload stub) and the
# one-wait-per-instruction legalizer this walrus build requires.
# ---------------------------------------------------------------------------

def _setup_axon_env():
    if 'antenv.axon_hooks' not in sys.modules:
        mod = types.ModuleType('antenv.axon_hooks')
        mod._hook = None
        mod.set_axon_ntff_profile_hook = lambda h: setattr(mod, '_hook', h)
        mod.get_axon_ntff_profile_hook = lambda: mod._hook
        sys.modules['antenv.axon_hooks'] = mod
        try:
            import antenv
            antenv.axon_hooks = mod
        except ImportError:
            pass
        try:
            from trn_agent_boot.trn_boot import _ntff_profile_via_ctypes
            mod._hook = _ntff_profile_via_ctypes('/opt/axon/libaxon_pjrt.so')
        except Exception:
            pass
    import concourse.bass_utils as bu
    bu.upload_artifacts = lambda tmpdir: 'file://' + str(tmpdir)


def _legalize_waits(nc):
    """walrus accepts at most ONE sync wait per engine instruction (2 for
    EventSemaphore). Split extras onto preceding same-engine NoOps."""
    import concourse.mybir as mybir
    n_split = 0
    for fn in nc.m.functions:
        for bb in fn.blocks:
            insts = bb.instructions
            out = []
            for inst in insts:
                si = inst.sync_info
                ow = list(si.on_wait) if si is not None and si.on_wait else []
                cap = 2 if inst.opcode == "EventSemaphore" else 1
                if len(ow) > cap:
                    keep = ow[-cap:]
                    for k, w in enumerate(ow[:-cap]):
                        nop = mybir.InstNoOp(
                            name=f"{inst.name}-wsplit{k}",
                            engine=inst.engine,
                            ins=[],
                            outs=[],
                            sync_info=mybir.SyncInfo(on_wait=[w], on_update=[]),
                        )
                        out.append(nop)
                        n_split += 1
                    inst.sync_info = mybir.SyncInfo(
                        on_wait=keep,
                        on_update=list(si.on_update) if si.on_update else [],
                    )
                out.append(inst)
            insts[:] = out
    return n_split


# ---------------------------------------------------------------------------
# device program
# ---------------------------------------------------------------------------

_CACHE = {}
last_results = None  # BassKernelResults of the most recent run (for test.py)


def _build_program():
    from contextlib import ExitStack
    import concourse.bass as bass
    import concourse.mybir as mybir
    import concourse.tile as tile

    F32 = mybir.dt.float32
    F32R = mybir.dt.float32r
    B16 = mybir.dt.bfloat16
    F8E3 = mybir.dt.float8e3

    nc = bass.Bass()

    # xt: host-pretransposed input, [p, c, k, b] = x[b, 128k+p, c], bf16
    xtd = nc.declare_dram_parameter("xtd", [128, C, 2, B_LOC], B16,
                                    isOutput=False)
    w1t = nc.declare_dram_parameter("w1t", [128, C, 2, 2, 128], B16,
                                    isOutput=False)
    w2t = nc.declare_dram_parameter("w2t", [128, C * 2 * 2 * 128], B16,
                                    isOutput=False)
    w3m = nc.declare_dram_parameter("w3m", [128, C * 2 * C], B16,
                                    isOutput=False)
    b1s = nc.declare_dram_parameter("b1s", [128, C, 2], F32, isOutput=False)
    b2s = nc.declare_dram_parameter("b2s", [128, C, 2], F32, isOutput=False)
    # out[k, c, b]: partial per k-half; host sums the two halves.
    out = nc.declare_dram_parameter("out", [2, C, B_LOC], F32, isOutput=True)

    NSEC = len(SECTIONS)
    SEC = SECTIONS[0]

    with ExitStack() as ctx:
        tc = ctx.enter_context(tile.TileContext(nc))

        consts = ctx.enter_context(tc.tile_pool(name="consts", bufs=1))
        wtmp_pool = ctx.enter_context(tc.tile_pool(name="wtmp", bufs=1))
        xt_pool = ctx.enter_context(tc.tile_pool(name="xt", bufs=2))
        h1_pool = ctx.enter_context(tc.tile_pool(name="h1p", bufs=3))
        h2_pool = ctx.enter_context(tc.tile_pool(name="h2p", bufs=11))
        out_pool = ctx.enter_context(tc.tile_pool(name="outp", bufs=1))

        ps_g = ctx.enter_context(
            tc.tile_pool(name="ps_g", bufs=6, space="PSUM"))
        # L3 partial-sum banks: k=0 accumulates in col-group 0 (rows 0:19),
        # k=1 in col-group 1 (rows 32:51) of a second bank, so the two
        # narrow matmuls run concurrently on different PE column groups.
        ps_3a = ctx.enter_context(
            tc.tile_pool(name="ps_3a", bufs=1, space="PSUM"))
        ps_3b = ctx.enter_context(
            tc.tile_pool(name="ps_3b", bufs=1, space="PSUM"))

        # ---- X.T section slabs stream on the sync ring, self-paced by
        # the xt pool slots; everything else rides the scalar ring.
        # slab 0 loads immediately (split by class range so L1(c=0) can
        # start early); later slabs are emitted inside the previous
        # section's pipeline, gated on its progress, so their DMA doesn't
        # steal HBM bandwidth from the weight loads at startup.
        xts0 = xt_pool.tile([128, C, 2, SEC], B16, tag="xt")
        # Startup is DMA-latency-bound: the first L1 matmul needs only
        # w1[0:2] (0.26 MB) and x[0:2] (0.5 MB).  Stage tiny first chunks
        # and defer everything not needed in the first few classes so the
        # SDMA rails aren't clogged when the pipeline wants to start.
        nc.sync.dma_start(xts0[:, 0:1], xtd[:, 0:1, :, 0:SEC])
        slabs = [xts0]

        # Weight loads interleaved in class-consumption order: the class-c
        # pipeline needs w1[c] first, w2[c] two iterations later.
        w1sb = consts.tile([128, C, 2, 2, 128], B16)
        w2sb = consts.tile([128, C * 2 * 2 * 128], B16)
        w3sb = consts.tile([128, C * 2 * C], B16)
        b1sb = consts.tile([128, C, 2], F32)
        b2sb = consts.tile([128, C, 2], F32)
        NW2 = C * 2 * 2 * 128  # 9728
        W2C = NW2 // C  # w2 bytes-per-class stride in the flat view

        # All bulk loads ride the sync queue in class-need order; the
        # scalar queue stays free for ACT evictions (each dma_start
        # dispatch costs ~0.7us of engine-queue time).
        nc.sync.dma_start(w1sb[:, 0:1], w1t[:, 0:1])
        nc.sync.dma_start(b1sb[:], b1s[:])
        nc.sync.dma_start(b2sb[:], b2s[:])
        nc.sync.dma_start(xts0[:, 1:3], xtd[:, 1:3, :, 0:SEC])
        nc.sync.dma_start(w1sb[:, 1:3], w1t[:, 1:3])
        nc.sync.dma_start(w2sb[:, 0:1 * W2C], w2t[:, 0:1 * W2C])
        nc.sync.dma_start(xts0[:, 3:7], xtd[:, 3:7, :, 0:SEC])
        nc.sync.dma_start(w1sb[:, 3:7], w1t[:, 3:7])
        nc.sync.dma_start(w2sb[:, 1 * W2C:5 * W2C], w2t[:, 1 * W2C:5 * W2C])
        nc.sync.dma_start(w3sb[:], w3m[:])
        nc.sync.dma_start(xts0[:, 7:13], xtd[:, 7:13, :, 0:SEC])
        nc.sync.dma_start(w1sb[:, 7:13], w1t[:, 7:13])
        nc.sync.dma_start(w2sb[:, 5 * W2C:11 * W2C], w2t[:, 5 * W2C:11 * W2C])
        nc.sync.dma_start(xts0[:, 13:C], xtd[:, 13:C, :, 0:SEC])
        nc.sync.dma_start(w1sb[:, 13:C], w1t[:, 13:C])
        nc.sync.dma_start(w2sb[:, 11 * W2C:], w2t[:, 11 * W2C:])

        # PE warm-up burst: dummy matmuls while DMA fills SBUF, so the
        # HAM clock-gate reaches 8/8 before the first real GEMM issues.
        wu_l = consts.tile([128, 128], B16)
        nc.gpsimd.memset(wu_l[:], 0.0)
        wu_r = consts.tile([128, 512], B16)
        nc.gpsimd.memset(wu_r[:], 0.0)
        wu_ps = ps_g.tile([128, 512], mybir.dt.float32, tag="pg")
        for i in range(4):
            nc.tensor.matmul(wu_ps[:], wu_l[:], wu_r[:],
                             start=True, stop=True)

        w1v = w1sb[:]
        w2v = w2sb[:].rearrange("p (c k m j) -> p c k m j", c=C, k=2, m=2)
        w3v = w3sb[:].rearrange("p (c k q) -> p c k q", c=C, k=2)

        # Later slabs are allocated and DMA-started from inside the
        # previous section's class loop (gated on pipeline progress via
        # a dummy 1-element write) so their 5 MB transfers don't steal
        # HBM bandwidth from the startup-critical chunks.
        slabs.extend([None] * (NSEC - 1))

        for h in range(NSEC):
            xtv = slabs[h][:]
            sec0 = h * SEC
            ps3a = ps_3a.tile([128, SEC], mybir.dt.float32, tag="ps3a")
            ps3b = ps_3b.tile([128, SEC], mybir.dt.float32, tag="ps3b")
            h1_t = [None, None, None]
            h2_t = [None] * 11
            # Step order on the PE queue: L1(cc), L3-burst, L2(cc-2).
            # L2 lags two steps so h1 is fully evicted well before its
            # matmuls issue.  L3 runs every 4th step as a burst over 4
            # classes (q0 matmuls chained, q32 riding concurrently on
            # the second PE column group / second PSUM bank), so the
            # ~300ns full<->narrow array transition is paid once per
            # four classes instead of once per class.
            for cc in range(C + 4):
                if cc < C:
                    c = cc
                    h1 = h1_pool.tile([128, 2, SEC], B16, tag="h1")
                    h1_t[c % 3] = h1
                    for m in range(2):
                        pg = ps_g.tile([128, SEC], mybir.dt.float32,
                                       tag="pg")
                        for k in range(2):
                            nc.tensor.matmul(
                                pg[:], w1v[:, c, k, m, :],
                                xtv[:, c, k, :],
                                start=(k == 0), stop=(k == 1))
                        if m == 0:
                            nc.scalar.activation(
                                h1[:, m, :], pg[:],
                                mybir.ActivationFunctionType.Relu,
                                bias=b1sb[:, c, m:m+1])
                        else:
                            nc.vector.tensor_scalar(
                                h1[:, m, :], pg[:],
                                b1sb[:, c, m:m+1], 0.0,
                                mybir.AluOpType.add, mybir.AluOpType.max)
                    if cc == 8 and h + 1 < NSEC:
                        xts = xt_pool.tile([128, C, 2, SEC], B16,
                                           tag="xt", name=f"xts{h+1}")
                        # dummy write from h1 delays the slab DMA until
                        # this section is well underway (WAW ordering).
                        nc.gpsimd.tensor_copy(
                            xts[0:1, 0:1, 0:1, 0:1], h1[0:1, 0:1, 0:1])
                        nc.sync.dma_start(
                            xts[:],
                            xtd[:, :, :, (h + 1) * SEC:(h + 2) * SEC])
                        slabs[h + 1] = xts
                burst = {10: (0, 8), 21: (8, 19)}
                if cc in burst:
                    for c in range(*burst[cc]):
                        h2 = h2_t[c % 11]
                        nc.tensor.matmul(
                            ps3a[0:C], w3v[:, c, 0, :], h2[:, 0, :],
                            start=(c == 0), stop=(c == C - 1),
                            tile_position=(0, 0))
                        nc.tensor.matmul(
                            ps3b[32:32 + C], w3v[:, c, 1, :],
                            h2[:, 1, :],
                            start=(c == 0), stop=(c == C - 1),
                            tile_position=(0, 32))
                if 2 <= cc <= C + 1:
                    c = cc - 2
                    h1 = h1_t[c % 3]
                    h2 = h2_pool.tile([128, 2, SEC], B16, tag="h2")
                    h2_t[c % 11] = h2
                    for m in range(2):
                        pg = ps_g.tile([128, SEC], mybir.dt.float32,
                                       tag="pg")
                        for k in range(2):
                            nc.tensor.matmul(
                                pg[:], w2v[:, c, k, m, :],
                                h1[:, k, :],
                                start=(k == 0), stop=(k == 1))
                        if m == 0:
                            nc.scalar.activation(
                                h2[:, m, :], pg[:],
                                mybir.ActivationFunctionType.Relu,
                                bias=b2sb[:, c, m:m+1])
                        else:
                            nc.vector.tensor_scalar(
                                h2[:, m, :], pg[:],
                                b2sb[:, c, m:m+1], 0.0,
                                mybir.AluOpType.add, mybir.AluOpType.max)

            # Evict the two L3 partials (k0 at rows 0:19 of bank A, k1
            # at rows 32:51 of bank B); host sums them.  One copy on
            # ACT, one on DVE so neither eviction engine takes both.
            out_sb = out_pool.tile([64, SEC], F32, tag="osb")
            nc.scalar.copy(out_sb[0:C], ps3a[0:C])
            nc.vector.tensor_copy(out_sb[32:32 + C], ps3b[32:32 + C])
            nc.sync.dma_start(out[0, :, sec0:sec0 + SEC], out_sb[0:C])
            nc.sync.dma_start(out[1, :, sec0:sec0 + SEC],
                              out_sb[32:32 + C])

    _legalize_waits(nc)
    return nc


def _get_program():
    if 'nc' not in _CACHE:
        _setup_axon_env()
        _CACHE['nc'] = _build_program()
    return _CACHE['nc']


# ---------------------------------------------------------------------------
# host wrapper
# ---------------------------------------------------------------------------

def kernel(inputs, W1, b1, W2, b2, W3, b3):
    global last_results
    from concourse.bass_utils import run_bass_kernel_spmd

    nc = _get_program()

    inputs = np.asarray(inputs)
    W1 = np.asarray(W1, dtype=np.float32)
    b1 = np.asarray(b1, dtype=np.float32)
    W2 = np.asarray(W2, dtype=np.float32)
    b2 = np.asarray(b2, dtype=np.float32)
    W3 = np.asarray(W3, dtype=np.float32)
    b3 = np.asarray(b3, dtype=np.float32)

    # host-side layout prep for the shard: [p, c, k, b] = x[b, 128k+p, c]
    xbf = np.asarray(inputs).reshape(B, 2, 128, C).astype(BF16)
    xtd_full = np.ascontiguousarray(xbf.transpose(2, 3, 1, 0))

    # lhsT tiles: w1t[p, c, k, m, j] = W1[c, 128k+p, 128m+j]
    w1t = np.ascontiguousarray(
        W1.reshape(C, 2, 128, 2, 128).transpose(2, 0, 1, 3, 4)).astype(BF16)
    w2t = np.ascontiguousarray(
        W2.reshape(C, 2, 128, 2, 128).transpose(2, 0, 1, 3, 4)
    ).reshape(128, C * 2 * 2 * 128).astype(BF16)
    # w3m[p, c, k, c'] = (c'==c) * W3[c, 128k+p]
    w3m = np.zeros((128, C, 2, C), dtype=np.float32)
    for c in range(C):
        w3m[:, c, 0, c] = W3[c, :128]
        w3m[:, c, 1, c] = W3[c, 128:]
    w3m = w3m.reshape(128, C * 2 * C).astype(BF16)
    # b1s[p, c, m] = b1[c, 128m+p]
    b1s = np.ascontiguousarray(
        b1.reshape(C, 2, 128).transpose(2, 0, 1)).astype(np.float32)
    b2s = np.ascontiguousarray(
        b2.reshape(C, 2, 128).transpose(2, 0, 1)).astype(np.float32)

    core_ids = list(range(NCORES))
    in_maps = []
    for i in core_ids:
        in_maps.append({
            "xtd": np.ascontiguousarray(
                xtd_full[:, :, :, i * B_LOC:(i + 1) * B_LOC]),
            "w1t": w1t, "w2t": w2t, "w3m": w3m, "b1s": b1s, "b2s": b2s,
        })

    import os
    trace = bool(os.environ.get("BASS_TRACE"))
    res = run_bass_kernel_spmd(nc, in_maps, core_ids, trace=trace)
    last_results = res

    out_full = np.empty((B, C), dtype=np.float32)
    for i in core_ids:
        o2 = res.results[i]["out"]
        out_full[i * B_LOC:(i + 1) * B_LOC] = (o2[0] + o2[1]).T
    out_full += b3[None, :]
    return out_full



# revision 47
# speedup vs baseline: 1.0808x; 1.0029x over previous
"""Trainium2 Bass kernel for nn_CWDiscriminator (per-class 3-layer MLP).

reference:
    x = inputs.transpose(0, 2, 1)            # (B, C, F)
    h = relu(einsum('bcf,cfg->bcg', x, W1) + b1)
    h = relu(einsum('bcf,cfg->bcg', h, W2) + b2)
    out = einsum('bcf,cf->bc', h, W3) + b3   # (B, C)

B=16384, F=256, C=19. Data-parallel over B across 8 NeuronCores
(B_loc = 2048 per core). Per core, per class c:
  - inputs arrive as (B_loc, F*C) bf16 (host-cast); PE transpose-mode
    converts the f-strided slices into X.T tiles (f on partitions).
  - GEMM1 (bf16): H1.T = W1[c].T @ X.T  -> PSUM, evicted by ACT with
    fused bias+ReLU to fp32r.
  - GEMM2 (fp32r): H2.T = W2[c].T @ H1.T -> PSUM, evicted with
    bias+ReLU to fp32r (ACT/DVE split).
  - GEMM3 (fp32r): lhsT = W3 masked to column c (128, 19); all classes
    accumulate into one shared PSUM (19, b) region, so the final
    eviction is one op per half instead of per class.
Output per core is (C, B_loc) fp32; host transposes and adds b3.
"""

import sys
import types

import numpy as np
import ml_dtypes

B, F, C = 16384, 256, 19
NCORES = 8
B_LOC = B // NCORES          # 2048
SECTIONS = [512, 512, 512, 512]  # batch columns per PSUM-accum round
assert sum(SECTIONS) == 2048
NCHUNK = 512                 # matmul moving free dim (one fp32 PSUM bank)
FC = F * C                   # 4864

BF16 = ml_dtypes.bfloat16
F8E3 = ml_dtypes.float8_e3m4


# ---------------------------------------------------------------------------
# axon environment shims (NTFF profile hook + artifact upload stub) and the
# one-wait-per-instruction legalizer this walrus build requires.
# ---------------------------------------------------------------------------

def _setup_axon_env():
    if 'antenv.axon_hooks' not in sys.modules:
        mod = types.ModuleType('antenv.axon_hooks')
        mod._hook = None
        mod.set_axon_ntff_profile_hook = lambda h: setattr(mod, '_hook', h)
        mod.get_axon_ntff_profile_hook = lambda: mod._hook
        sys.modules['antenv.axon_hooks'] = mod
        try:
            import antenv
            antenv.axon_hooks = mod
        except ImportError:
            pass
        try:
            from trn_agent_boot.trn_boot import _ntff_profile_via_ctypes
            mod._hook = _ntff_profile_via_ctypes('/opt/axon/libaxon_pjrt.so')
        except Exception:
            pass
    import concourse.bass_utils as bu
    bu.upload_artifacts = lambda tmpdir: 'file://' + str(tmpdir)


def _legalize_waits(nc):
    """walrus accepts at most ONE sync wait per engine instruction (2 for
    EventSemaphore). Split extras onto preceding same-engine NoOps."""
    import concourse.mybir as mybir
    n_split = 0
    for fn in nc.m.functions:
        for bb in fn.blocks:
            insts = bb.instructions
            out = []
            for inst in insts:
                si = inst.sync_info
                ow = list(si.on_wait) if si is not None and si.on_wait else []
                cap = 2 if inst.opcode == "EventSemaphore" else 1
                if len(ow) > cap:
                    keep = ow[-cap:]
                    for k, w in enumerate(ow[:-cap]):
                        nop = mybir.InstNoOp(
                            name=f"{inst.name}-wsplit{k}",
                            engine=inst.engine,
                            ins=[],
                            outs=[],
                            sync_info=mybir.SyncInfo(on_wait=[w], on_update=[]),
                        )
                        out.append(nop)
                        n_split += 1
                    inst.sync_info = mybir.SyncInfo(
                        on_wait=keep,
                        on_update=list(si.on_update) if si.on_update else [],
                    )
                out.append(inst)
            insts[:] = out
    return n_split


# ---------------------------------------------------------------------------
# device program
# ---------------------------------------------------------------------------

_CACHE = {}
last_results = None  # BassKernelResults of the most recent run (for test.py)


def _build_program():
    from contextlib import ExitStack
    import concourse.bass as bass
    import concourse.mybir as mybir
    import concourse.tile as tile

    F32 = mybir.dt.float32
    F32R = mybir.dt.float32r
    B16 = mybir.dt.bfloat16
    F8E3 = mybir.dt.float8e3

    nc = bass.Bass()

    # xt: host-pretransposed input, [p, c, k, b] = x[b, 128k+p, c], bf16
    xtd = nc.declare_dram_parameter("xtd", [128, C, 2, B_LOC], B16,
                                    isOutput=False)
    w1t = nc.declare_dram_parameter("w1t", [128, C, 2, 2, 128], B16,
                                    isOutput=False)
    w2t = nc.declare_dram_parameter("w2t", [128, C * 2 * 2 * 128], B16,
                                    isOutput=False)
    w3m = nc.declare_dram_parameter("w3m", [128, C * 2 * C], B16,
                                    isOutput=False)
    b1s = nc.declare_dram_parameter("b1s", [128, C, 2], F32, isOutput=False)
    b2s = nc.declare_dram_parameter("b2s", [128, C, 2], F32, isOutput=False)
    # out[k, c, b]: partial per k-half; host sums the two halves.
    out = nc.declare_dram_parameter("out", [2, C, B_LOC], F32, isOutput=True)

    NSEC = len(SECTIONS)
    SEC = SECTIONS[0]

    with ExitStack() as ctx:
        tc = ctx.enter_context(tile.TileContext(nc))

        consts = ctx.enter_context(tc.tile_pool(name="consts", bufs=1))
        wtmp_pool = ctx.enter_context(tc.tile_pool(name="wtmp", bufs=1))
        xt_pool = ctx.enter_context(tc.tile_pool(name="xt", bufs=2))
        h1_pool = ctx.enter_context(tc.tile_pool(name="h1p", bufs=3))
        h2_pool = ctx.enter_context(tc.tile_pool(name="h2p", bufs=19))
        out_pool = ctx.enter_context(tc.tile_pool(name="outp", bufs=1))

        ps_g = ctx.enter_context(
            tc.tile_pool(name="ps_g", bufs=6, space="PSUM"))
        # L3 partial-sum banks: k=0 accumulates in col-group 0 (rows 0:19),
        # k=1 in col-group 1 (rows 32:51) of a second bank, so the two
        # narrow matmuls run concurrently on different PE column groups.
        ps_3a = ctx.enter_context(
            tc.tile_pool(name="ps_3a", bufs=1, space="PSUM"))
        ps_3b = ctx.enter_context(
            tc.tile_pool(name="ps_3b", bufs=1, space="PSUM"))

        # ---- X.T section slabs stream on the sync ring, self-paced by
        # the xt pool slots; everything else rides the scalar ring.
        # slab 0 loads immediately (split by class range so L1(c=0) can
        # start early); later slabs are emitted inside the previous
        # section's pipeline, gated on its progress, so their DMA doesn't
        # steal HBM bandwidth from the weight loads at startup.
        xts0 = xt_pool.tile([128, C, 2, SEC], B16, tag="xt")
        # Startup is DMA-latency-bound: the first L1 matmul needs only
        # w1[0:2] (0.26 MB) and x[0:2] (0.5 MB).  Stage tiny first chunks
        # and defer everything not needed in the first few classes so the
        # SDMA rails aren't clogged when the pipeline wants to start.
        nc.sync.dma_start(xts0[:, 0:1], xtd[:, 0:1, :, 0:SEC])
        slabs = [xts0]

        # Weight loads interleaved in class-consumption order: the class-c
        # pipeline needs w1[c] first, w2[c] two iterations later.
        w1sb = consts.tile([128, C, 2, 2, 128], B16)
        w2sb = consts.tile([128, C * 2 * 2 * 128], B16)
        w3sb = consts.tile([128, C * 2 * C], B16)
        b1sb = consts.tile([128, C, 2], F32)
        b2sb = consts.tile([128, C, 2], F32)
        NW2 = C * 2 * 2 * 128  # 9728
        W2C = NW2 // C  # w2 bytes-per-class stride in the flat view

        # All bulk loads ride the sync queue in class-need order; the
        # scalar queue stays free for ACT evictions (each dma_start
        # dispatch costs ~0.7us of engine-queue time).
        nc.sync.dma_start(w1sb[:, 0:1], w1t[:, 0:1])
        nc.sync.dma_start(b1sb[:], b1s[:])
        nc.sync.dma_start(b2sb[:], b2s[:])
        nc.sync.dma_start(xts0[:, 1:3], xtd[:, 1:3, :, 0:SEC])
        nc.sync.dma_start(w1sb[:, 1:3], w1t[:, 1:3])
        nc.sync.dma_start(w2sb[:, 0:1 * W2C], w2t[:, 0:1 * W2C])
        nc.sync.dma_start(xts0[:, 3:7], xtd[:, 3:7, :, 0:SEC])
        nc.sync.dma_start(w1sb[:, 3:7], w1t[:, 3:7])
        nc.sync.dma_start(w2sb[:, 1 * W2C:5 * W2C], w2t[:, 1 * W2C:5 * W2C])
        nc.sync.dma_start(w3sb[:], w3m[:])
        nc.sync.dma_start(xts0[:, 7:13], xtd[:, 7:13, :, 0:SEC])
        nc.sync.dma_start(w1sb[:, 7:13], w1t[:, 7:13])
        nc.sync.dma_start(w2sb[:, 5 * W2C:11 * W2C], w2t[:, 5 * W2C:11 * W2C])
        nc.sync.dma_start(xts0[:, 13:C], xtd[:, 13:C, :, 0:SEC])
        nc.sync.dma_start(w1sb[:, 13:C], w1t[:, 13:C])
        nc.sync.dma_start(w2sb[:, 11 * W2C:], w2t[:, 11 * W2C:])

        # PE warm-up burst: dummy matmuls while DMA fills SBUF, so the
        # HAM clock-gate reaches 8/8 before the first real GEMM issues.
        wu_l = consts.tile([128, 128], B16)
        nc.gpsimd.memset(wu_l[:], 0.0)
        wu_r = consts.tile([128, 512], B16)
        nc.gpsimd.memset(wu_r[:], 0.0)
        wu_ps = ps_g.tile([128, 512], mybir.dt.float32, tag="pg")
        for i in range(4):
            nc.tensor.matmul(wu_ps[:], wu_l[:], wu_r[:],
                             start=True, stop=True)

        w1v = w1sb[:]
        w2v = w2sb[:].rearrange("p (c k m j) -> p c k m j", c=C, k=2, m=2)
        w3v = w3sb[:].rearrange("p (c k q) -> p c k q", c=C, k=2)

        # Later slabs are allocated and DMA-started from inside the
        # previous section's class loop (gated on pipeline progress via
        # a dummy 1-element write) so their 5 MB transfers don't steal
        # HBM bandwidth from the startup-critical chunks.
        slabs.extend([None] * (NSEC - 1))

        for h in range(NSEC):
            xtv = slabs[h][:]
            sec0 = h * SEC
            ps3a = ps_3a.tile([128, SEC], mybir.dt.float32, tag="ps3a")
            ps3b = ps_3b.tile([128, SEC], mybir.dt.float32, tag="ps3b")
            h1_t = [None, None, None]
            h2_t = [None] * 19
            # Step order on the PE queue: L1(cc), L3-burst, L2(cc-2).
            # L2 lags two steps so h1 is fully evicted well before its
            # matmuls issue.  L3 runs every 4th step as a burst over 4
            # classes (q0 matmuls chained, q32 riding concurrently on
            # the second PE column group / second PSUM bank), so the
            # ~300ns full<->narrow array transition is paid once per
            # four classes instead of once per class.
            for cc in range(C + 4):
                if cc < C:
                    c = cc
                    h1 = h1_pool.tile([128, 2, SEC], B16, tag="h1")
                    h1_t[c % 3] = h1
                    for m in range(2):
                        pg = ps_g.tile([128, SEC], mybir.dt.float32,
                                       tag="pg")
                        for k in range(2):
                            nc.tensor.matmul(
                                pg[:], w1v[:, c, k, m, :],
                                xtv[:, c, k, :],
                                start=(k == 0), stop=(k == 1))
                        if m == 0:
                            nc.scalar.activation(
                                h1[:, m, :], pg[:],
                                mybir.ActivationFunctionType.Relu,
                                bias=b1sb[:, c, m:m+1])
                        else:
                            nc.vector.tensor_scalar(
                                h1[:, m, :], pg[:],
                                b1sb[:, c, m:m+1], 0.0,
                                mybir.AluOpType.add, mybir.AluOpType.max)
                    if cc == 8 and h + 1 < NSEC:
                        xts = xt_pool.tile([128, C, 2, SEC], B16,
                                           tag="xt", name=f"xts{h+1}")
                        # dummy write from h1 delays the slab DMA until
                        # this section is well underway (WAW ordering).
                        nc.gpsimd.tensor_copy(
                            xts[0:1, 0:1, 0:1, 0:1], h1[0:1, 0:1, 0:1])
                        nc.sync.dma_start(
                            xts[:],
                            xtd[:, :, :, (h + 1) * SEC:(h + 2) * SEC])
                        slabs[h + 1] = xts
                burst = {21: (0, 19)}
                if cc in burst:
                    for c in range(*burst[cc]):
                        h2 = h2_t[c % 19]
                        nc.tensor.matmul(
                            ps3a[0:C], w3v[:, c, 0, :], h2[:, 0, :],
                            start=(c == 0), stop=(c == C - 1),
                            tile_position=(0, 0))
                        nc.tensor.matmul(
                            ps3b[32:32 + C], w3v[:, c, 1, :],
                            h2[:, 1, :],
                            start=(c == 0), stop=(c == C - 1),
                            tile_position=(0, 32))
                if 2 <= cc <= C + 1:
                    c = cc - 2
                    h1 = h1_t[c % 3]
                    h2 = h2_pool.tile([128, 2, SEC], B16, tag="h2")
                    h2_t[c % 19] = h2
                    for m in range(2):
                        pg = ps_g.tile([128, SEC], mybir.dt.float32,
                                       tag="pg")
                        for k in range(2):
                            nc.tensor.matmul(
                                pg[:], w2v[:, c, k, m, :],
                                h1[:, k, :],
                                start=(k == 0), stop=(k == 1))
                        if m == 0:
                            nc.scalar.activation(
                                h2[:, m, :], pg[:],
                                mybir.ActivationFunctionType.Relu,
                                bias=b2sb[:, c, m:m+1])
                        else:
                            nc.vector.tensor_scalar(
                                h2[:, m, :], pg[:],
                                b2sb[:, c, m:m+1], 0.0,
                                mybir.AluOpType.add, mybir.AluOpType.max)

            # Evict the two L3 partials (k0 at rows 0:19 of bank A, k1
            # at rows 32:51 of bank B); host sums them.  One copy on
            # ACT, one on DVE so neither eviction engine takes both.
            out_sb = out_pool.tile([64, SEC], F32, tag="osb")
            nc.scalar.copy(out_sb[0:C], ps3a[0:C])
            nc.vector.tensor_copy(out_sb[32:32 + C], ps3b[32:32 + C])
            nc.sync.dma_start(out[0, :, sec0:sec0 + SEC], out_sb[0:C])
            nc.sync.dma_start(out[1, :, sec0:sec0 + SEC],
                              out_sb[32:32 + C])

    _legalize_waits(nc)
    return nc


def _get_program():
    if 'nc' not in _CACHE:
        _setup_axon_env()
        _CACHE['nc'] = _build_program()
    return _CACHE['nc']


# ---------------------------------------------------------------------------
# host wrapper
# ---------------------------------------------------------------------------

def kernel(inputs, W1, b1, W2, b2, W3, b3):
    global last_results
    from concourse.bass_utils import run_bass_kernel_spmd

    nc = _get_program()

    inputs = np.asarray(inputs)
    W1 = np.asarray(W1, dtype=np.float32)
    b1 = np.asarray(b1, dtype=np.float32)
    W2 = np.asarray(W2, dtype=np.float32)
    b2 = np.asarray(b2, dtype=np.float32)
    W3 = np.asarray(W3, dtype=np.float32)
    b3 = np.asarray(b3, dtype=np.float32)

    # host-side layout prep for the shard: [p, c, k, b] = x[b, 128k+p, c]
    xbf = np.asarray(inputs).reshape(B, 2, 128, C).astype(BF16)
    xtd_full = np.ascontiguousarray(xbf.transpose(2, 3, 1, 0))

    # lhsT tiles: w1t[p, c, k, m, j] = W1[c, 128k+p, 128m+j]
    w1t = np.ascontiguousarray(
        W1.reshape(C, 2, 128, 2, 128).transpose(2, 0, 1, 3, 4)).astype(BF16)
    w2t = np.ascontiguousarray(
        W2.reshape(C, 2, 128, 2, 128).transpose(2, 0, 1, 3, 4)
    ).reshape(128, C * 2 * 2 * 128).astype(BF16)
    # w3m[p, c, k, c'] = (c'==c) * W3[c, 128k+p]
    w3m = np.zeros((128, C, 2, C), dtype=np.float32)
    for c in range(C):
        w3m[:, c, 0, c] = W3[c, :128]
        w3m[:, c, 1, c] = W3[c, 128:]
    w3m = w3m.reshape(128, C * 2 * C).astype(BF16)
    # b1s[p, c, m] = b1[c, 128m+p]
    b1s = np.ascontiguousarray(
        b1.reshape(C, 2, 128).transpose(2, 0, 1)).astype(np.float32)
    b2s = np.ascontiguousarray(
        b2.reshape(C, 2, 128).transpose(2, 0, 1)).astype(np.float32)

    core_ids = list(range(NCORES))
    in_maps = []
    for i in core_ids:
        in_maps.append({
            "xtd": np.ascontiguousarray(
                xtd_full[:, :, :, i * B_LOC:(i + 1) * B_LOC]),
            "w1t": w1t, "w2t": w2t, "w3m": w3m, "b1s": b1s, "b2s": b2s,
        })

    import os
    trace = bool(os.environ.get("BASS_TRACE"))
    res = run_bass_kernel_spmd(nc, in_maps, core_ids, trace=trace)
    last_results = res

    out_full = np.empty((B, C), dtype=np.float32)
    for i in core_ids:
        o2 = res.results[i]["out"]
        out_full[i * B_LOC:(i + 1) * B_LOC] = (o2[0] + o2[1]).T
    out_full += b3[None, :]
    return out_full

